# revision 50
# baseline (speedup 1.0000x reference)
"""AutoBoxGraphAttention Trainium2 kernel (optimized).

Data-parallel over batch: core b handles image b (B=8, one per NeuronCore).
The only cross-core communication is a 4KB AllReduce of BatchNorm partials.

Key layout/scheduling choices vs the naive version:
  - qkv biases folded into the PSUM->SBUF copies (tensor_tensor add with
    host-precomputed partition-broadcast bias tiles). No K=1 bias matmuls.
  - edge bias folded into the QK^T contraction as a 65th K row: qka has a
    65th partition whose q-slots hold edge[h,i,w] and k-slots hold 1.0.
  - v conv processes two image rows per matmul (N=128, full PE width).
  - S uses transposed free layout S[W', w*64+i] so the softmax reduction
    over i is contiguous; exp() reads PSUM directly (no S copy).
  - agg returns to conv world via SBUF->SBUF DMAs (no DRAM bounce, no
    readback), issue alternating sync/gpsimd queues.
  - fusion conv runs as two passes: the x half (independent of attention)
    right after attention to hide the agg DMA drain, then the agg half
    accumulated into yt with DVE adds.

Spatial layout "w65": each row padded to 65 elements with a LEADING zero
(index d*65 holds 0, data at d*65+1 .. +64, plus 4 trailing zeros; total
4164). A 3x3 tap (dy,dx) then reads a plain strided AP at offset
(r+dy-1)*65 + dx; horizontal SAME-padding is automatic, vertical padding
via per-tap row clamping.
"""

import os
import sys

for _p in ("/opt/trn_rl_repo", os.path.dirname(os.path.abspath(__file__))):
    if _p not in sys.path:
        sys.path.insert(0, _p)

import numpy as np
import ml_dtypes

import concourse.tile as _tile_mod


def _apply_toolchain_patches():
    """This container's walrus accepts at most ONE sync-wait per
    instruction; Tile's exit drain and scheduler attach several. Split the
    exit drain into single-wait drains, and post-process the module to
    hoist extra waits onto same-engine NoOps."""
    import concourse.mybir as mybir

    def _split_drain_and_barrier(self, tick_clock, wait_clock):
        from concourse.tile import ScopedClock
        nc = self.nc
        drain_inst = nc.sync.drain()
        wait_clock.add_sem_waits(
            drain_inst.ins, ScopedClock({None: tick_clock.global_clock}))
        si = drain_inst.ins.sync_info
        if si is not None and len(si.on_wait) > 1:
            waits = list(si.on_wait)
            drain_inst.ins.sync_info = type(si)(
                on_wait=waits[:1], on_update=list(si.on_update))
            for w in waits[1:]:
                d2 = nc.sync.drain()
                si2 = d2.ins.sync_info
                if si2 is None:
                    d2.ins.sync_info = type(si)(on_wait=[w], on_update=[])
                else:
                    d2.ins.sync_info = type(si2)(
                        on_wait=list(si2.on_wait) + [w],
                        on_update=list(si2.on_update))
        nc.all_engine_barrier()
        assert self.sems is not None
        popped = nc._tile_sem_poison_stack.pop()
        assert popped is self._sem_poison
        nc.clear_and_free_semaphores(list(self.sems.allocated().values()))
        nc.all_engine_barrier()

    _tile_mod.TileContext._drain_and_barrier = _split_drain_and_barrier


def _split_multi_waits(nc):
    import concourse.mybir as mybir
    n_split = 0
    for fn in nc.m.functions:
        for bb in fn.blocks:
            insts = list(bb.instructions)
            out = []
            changed = False
            for inst in insts:
                si = inst.sync_info
                if si is not None and len(si.on_wait) > 1:
                    waits = list(si.on_wait)
                    for w in waits[:-1]:
                        nop = mybir.InstNoOp(
                            name=f"{inst.name}-wsplit{n_split}",
                            engine=inst.engine, bass_nofuse=True)
                        nop.sync_info = mybir.SyncInfo(on_wait=[w], on_update=[])
                        out.append(nop)
                        n_split += 1
                    inst.sync_info = type(si)(
                        on_wait=[waits[-1]], on_update=list(si.on_update))
                    changed = True
                out.append(inst)
            if changed:
                bb.instructions = out
    return n_split


_apply_toolchain_patches()

import concourse.bass as bass  # noqa: E402
import concourse.tile as tile  # noqa: E402
from concourse import mybir  # noqa: E402
from concourse.bass_utils import run_bass_kernel_spmd  # noqa: E402

F32 = mybir.dt.float32
BF16 = mybir.dt.bfloat16

B, C, H, W = 8, 512, 64, 64
NH, HD = 8, 64
EPS = 1e-5
W65 = 65
SP65 = 64 * W65 + 4  # 4164
SP = 4096
N_CORES = 8

LAST_RESULTS = None


def _ap(t, offset, dims):
    return bass.AP(tensor=t.tensor, offset=t.offset + offset, ap=[t.ap[0]] + dims)


def _slice_part(t, p0, np_, offset, dims):
    # tile[p0:p0+np_] then rebuild free dims
    sub = t[p0:p0 + np_]
    return bass.AP(tensor=sub.tensor, offset=sub.offset + offset,
                   ap=[sub.ap[0]] + dims)


TAPS = [(1, 0), (1, 1), (1, 2), (0, 0), (0, 1), (0, 2), (2, 0), (2, 1), (2, 2)]


def conv3x3_chunk(nc, psum, w_tiles, src_tiles, r0, n_ktiles, co=128):
    """3x3 conv, one 8-row chunk, accumulated into psum (co, 8*64).
    w_tiles[(tap_idx, kt)] = lhsT (K, co); src_tiles[kt] = w65 tile.
    dy=1 taps first (full coverage -> start=True clears the bank)."""
    mms = []
    for dy, dx in TAPS:
        t_i = dy * 3 + dx  # host weight tap order is (ky, kx) row-major
        a = max(r0, 1 - dy)
        b = min(r0 + 8, 65 - dy)
        nr = b - a
        if nr <= 0:
            continue
        for kt in range(n_ktiles):
            mms.append((t_i, kt, a, nr, dy, dx))
    for j, (t_i, kt, a, nr, dy, dx) in enumerate(mms):
        src = src_tiles[kt]
        in_ap = _ap(src, (a + dy - 1) * W65 + dx, [[W65, nr], [1, 64]])
        nc.tensor.matmul(
            psum[0:co, (a - r0) * 64:(a - r0 + nr) * 64], w_tiles[(t_i, kt)],
            in_ap, start=(j == 0), stop=(j == len(mms) - 1),
            skip_group_check=True)


def build_program():
    nc = bass.Bass(trn_type="TRN2", num_devices=N_CORES)

    d_x65 = nc.dram_tensor("x65", [C, SP65], BF16, kind="ExternalInput")
    d_xT = nc.dram_tensor("xT", [C, SP], BF16, kind="ExternalInput")
    d_wqkT = nc.dram_tensor("wqkT", [C, 1024], BF16, kind="ExternalInput")
    d_wvT = nc.dram_tensor("wvT", [C, 512], BF16, kind="ExternalInput")
    d_bqkB = nc.dram_tensor("bqkB", [1, 1024], BF16, kind="ExternalInput")
    d_bvB = nc.dram_tensor("bvB", [1, 512], BF16, kind="ExternalInput")
    d_bw1 = nc.dram_tensor("bw1", [9, C, 64], BF16, kind="ExternalInput")
    d_bb1 = nc.dram_tensor("bb1", [64, 1], F32, kind="ExternalInput")
    d_bw2 = nc.dram_tensor("bw2", [64, 4], BF16, kind="ExternalInput")
    d_bb2 = nc.dram_tensor("bb2", [4, 1], F32, kind="ExternalInput")
    d_ew1 = nc.dram_tensor("ew1", [9, 4, 64], BF16, kind="ExternalInput")
    d_eb1 = nc.dram_tensor("eb1", [64, 1], F32, kind="ExternalInput")
    d_ew2 = nc.dram_tensor("ew2", [64, 8], BF16, kind="ExternalInput")
    d_eb2 = nc.dram_tensor("eb2", [8, 1], F32, kind="ExternalInput")
    d_gng = nc.dram_tensor("gng", [64, 1], F32, kind="ExternalInput")
    d_gnb = nc.dram_tensor("gnb", [64, 1], F32, kind="ExternalInput")
    d_gmat = nc.dram_tensor("gmat", [64, 8], F32, kind="ExternalInput")
    d_fwT = nc.dram_tensor("fwT", [9, 8, 4, 128, 128], BF16, kind="ExternalInput")
    d_fb = nc.dram_tensor("fb", [C, 1], F32, kind="ExternalInput")
    d_bng = nc.dram_tensor("bng", [C, 1], F32, kind="ExternalInput")
    d_bnb = nc.dram_tensor("bnb", [C, 1], F32, kind="ExternalInput")
    d_y = nc.dram_tensor("y", [C, SP], F32, kind="ExternalOutput")

    AF = mybir.ActivationFunctionType

    with tile.TileContext(nc) as tc:
        with tc.tile_pool(name="glob", bufs=1) as glob, \
             tc.tile_pool(name="psA", bufs=4, space="PSUM") as psA, \
             tc.tile_pool(name="psB", bufs=4, space="PSUM") as psB, \
             tc.tile_pool(name="dram", bufs=1, space="DRAM") as dram:

            x65 = [glob.tile([128, SP65], BF16, name=f"x65_{k}")
                   for k in range(4)]
            agg65 = [glob.tile([128, SP65], BF16, name=f"agg65_{k}")
                     for k in range(4)]
            ones1 = glob.tile([1, 128], BF16)
            fb = glob.tile([128, 4], F32)
            nc.sync.dma_start(
                out=fb, in_=d_fb[:].rearrange("(a b) c -> b (a c)", a=4))
            for k in range(4):
                nc.sync.dma_start(out=x65[k], in_=d_x65[k * 128:(k + 1) * 128, :])
                nc.gpsimd.memset(agg65[k], 0.0)
            nc.vector.memset(ones1, 1.0)
            fw0 = {}
            for t_i in range(9):
                for kt in range(4):
                    fw0[(t_i, kt)] = glob.tile([128, 128], BF16,
                                               name=f"fw0_{t_i}_{kt}")

            # ================= Phases 1-2 pool =================
            with tc.tile_pool(name="ph12", bufs=1) as ph12:
                xT = [ph12.tile([128, SP], BF16, name=f"xT_{k}")
                      for k in range(4)]
                wqkT = [ph12.tile([128, 1024], BF16, name=f"wqkT_{kt}")
                        for kt in range(4)]
                wvT = [ph12.tile([128, 512], BF16, name=f"wvT_{kt}")
                       for kt in range(4)]
                for kt in range(4):
                    nc.gpsimd.dma_start(out=wqkT[kt],
                                        in_=d_wqkT[kt * 128:(kt + 1) * 128, :])
                    nc.gpsimd.dma_start(out=wvT[kt],
                                        in_=d_wvT[kt * 128:(kt + 1) * 128, :])
                bqkB = ph12.tile([1, 1024], BF16)
                nc.gpsimd.dma_start(out=bqkB, in_=d_bqkB[:])
                bvB = ph12.tile([1, 512], BF16)
                nc.gpsimd.dma_start(out=bvB, in_=d_bvB[:])
                edge_flT = ph12.tile([8, SP], BF16)

                # ---------- Phase 1: box_net + edge_net ----------
                with tc.tile_pool(name="p1", bufs=1) as p1:
                    for k in range(4):
                        nc.gpsimd.dma_start(out=xT[k],
                                            in_=d_xT[k * 128:(k + 1) * 128, :])
                    bw1 = {}
                    for t_i in range(9):
                        for kt in range(4):
                            w = p1.tile([128, 64], BF16, name=f"bw1_{t_i}_{kt}")
                            nc.sync.dma_start(
                                out=w, in_=d_bw1[t_i, kt * 128:(kt + 1) * 128, :])
                            bw1[(t_i, kt)] = w
                    bb1 = p1.tile([64, 1], F32)
                    nc.sync.dma_start(out=bb1, in_=d_bb1[:])
                    bw2 = p1.tile([64, 4], BF16)
                    nc.sync.dma_start(out=bw2, in_=d_bw2[:])
                    bb2 = p1.tile([4, 1], F32)
                    nc.sync.dma_start(out=bb2, in_=d_bb2[:])
                    ew1 = {}
                    for t_i in range(9):
                        w = p1.tile([4, 64], BF16, name=f"ew1_{t_i}")
                        nc.sync.dma_start(out=w, in_=d_ew1[t_i, :, :])
                        ew1[(t_i, 0)] = w
                    eb1 = p1.tile([64, 1], F32)
                    nc.sync.dma_start(out=eb1, in_=d_eb1[:])
                    ew2 = p1.tile([64, 8], BF16)
                    nc.sync.dma_start(out=ew2, in_=d_ew2[:])
                    eb2 = p1.tile([8, 1], F32)
                    nc.sync.dma_start(out=eb2, in_=d_eb2[:])
                    gng = p1.tile([64, 1], F32)
                    nc.sync.dma_start(out=gng, in_=d_gng[:])
                    gnb = p1.tile([64, 1], F32)
                    nc.sync.dma_start(out=gnb, in_=d_gnb[:])
                    gmat = p1.tile([64, 8], F32)
                    nc.sync.dma_start(out=gmat, in_=d_gmat[:])
                    for (t_i, kt), w in fw0.items():
                        nc.sync.dma_start(out=w, in_=d_fwT[t_i, kt, 0, :, :])

                    box1 = p1.tile([64, SP65], BF16)
                    nc.vector.memset(box1, 0.0)
                    for ch in range(8):
                        pb = psA.tile([128, 512], F32, tag="conv", name=f"pb_{ch}")
                        conv3x3_chunk(nc, pb, bw1, x65, ch * 8, 4, co=64)
                        nc.scalar.activation(
                            out=_slice_part(box1, 0, 64, ch * 8 * W65 + 1,
                                            [[W65, 8], [1, 64]]),
                            in_=pb[0:64, :], func=AF.Gelu, bias=bb1, scale=1.0)

                    boxes = p1.tile([4, SP65], BF16)
                    nc.vector.memset(boxes, 0.0)
                    for ch in range(8):
                        pb2 = psA.tile([128, 512], F32, tag="conv", name=f"pb2_{ch}")
                        nc.tensor.matmul(
                            pb2[0:4, :], bw2,
                            _ap(box1, ch * 8 * W65 + 1, [[W65, 8], [1, 64]]),
                            start=True, stop=True)
                        nc.scalar.activation(
                            out=_slice_part(boxes, 0, 4, ch * 8 * W65 + 1,
                                            [[W65, 8], [1, 64]]),
                            in_=pb2[0:4, :], func=AF.Sigmoid, bias=bb2, scale=1.0)

                    e1 = p1.tile([64, SP], F32)
                    for ch in range(8):
                        pe = psA.tile([128, 512], F32, tag="conv", name=f"pe_{ch}")
                        conv3x3_chunk(nc, pe, ew1, [boxes], ch * 8, 1, co=64)
                        nc.scalar.activation(
                            out=e1[:, ch * 512:(ch + 1) * 512], in_=pe[0:64, :],
                            func=AF.Identity, bias=eb1, scale=1.0)

                    stats = p1.tile([64, 8, 6], F32)
                    for j in range(8):
                        nc.vector.bn_stats(out=stats[:, j, :],
                                           in_=e1[:, j * 512:(j + 1) * 512])
                    mv = p1.tile([64, 2], F32)
                    nc.vector.bn_aggr(out=mv, in_=stats)
                    ex2 = p1.tile([64, 2], F32)
                    nc.vector.tensor_copy(out=ex2[:, 0:1], in_=mv[:, 0:1])
                    nc.vector.tensor_mul(out=ex2[:, 1:2], in0=mv[:, 0:1],
                                         in1=mv[:, 0:1])
                    nc.vector.tensor_add(out=ex2[:, 1:2], in0=ex2[:, 1:2],
                                         in1=mv[:, 1:2])
                    gs_ps = psB.tile([8, 2], F32, tag="att", name="gs_ps")
                    nc.tensor.matmul(gs_ps, gmat, ex2, start=True, stop=True)
                    gs = p1.tile([8, 2], F32)
                    nc.scalar.activation(out=gs, in_=gs_ps, func=AF.Copy,
                                         bias=0.0, scale=1.0 / 8.0)
                    gvar = p1.tile([8, 1], F32)
                    eps8 = p1.tile([8, 1], F32)
                    nc.vector.memset(eps8, float(EPS))
                    nc.vector.tensor_mul(out=gvar, in0=gs[:, 0:1], in1=gs[:, 0:1])
                    nc.vector.tensor_sub(out=gvar, in0=gs[:, 1:2], in1=gvar)
                    nc.scalar.activation(out=gvar, in_=gvar, func=AF.Sqrt,
                                         bias=eps8, scale=1.0)
                    nc.vector.reciprocal(out=gvar, in_=gvar)
                    gmr = p1.tile([8, 2], F32)
                    nc.vector.tensor_copy(out=gmr[:, 0:1], in_=gs[:, 0:1])
                    nc.vector.tensor_copy(out=gmr[:, 1:2], in_=gvar)
                    cmr = p1.tile([64, 2], F32)
                    src_bc = bass.AP(tensor=gmr.tensor, offset=gmr.offset,
                                     ap=[[gmr.ap[0][0], 8], [0, 8], [1, 2]])
                    nc.sync.dma_start(out=cmr, in_=src_bc)
                    gsc = p1.tile([64, 1], F32)
                    nc.vector.tensor_mul(out=gsc, in0=cmr[:, 1:2], in1=gng)
                    gsh = p1.tile([64, 1], F32)
                    nc.vector.tensor_mul(out=gsh, in0=cmr[:, 0:1], in1=gsc)
                    nc.vector.tensor_sub(out=gsh, in0=gnb, in1=gsh)
                    e1g = p1.tile([64, SP], BF16)
                    nc.scalar.activation(out=e1g, in_=e1, func=AF.Gelu,
                                         bias=gsh, scale=gsc)
                    # edge conv2 with transposed moving AP -> edge_flT[h, w*64+i]
                    for wb in range(8):
                        pe2 = psA.tile([128, 512], F32, tag="conv",
                                       name=f"pe2_{wb}")
                        nc.tensor.matmul(
                            pe2[0:8, :], ew2,
                            _ap(e1g, wb * 8, [[1, 8], [64, 64]]),
                            start=True, stop=True)
                        nc.scalar.activation(
                            out=edge_flT[:, wb * 512:(wb + 1) * 512],
                            in_=pe2[0:8, :], func=AF.Identity, bias=eb2,
                            scale=1.0)

                # ---------- Phase 2: qkv conv + attention ----------
                with tc.tile_pool(name="p2", bufs=1) as p2:
                    n_agg_dma = 0
                    n_drain = 0

                    def drain(out, in_):
                        nonlocal n_drain
                        n_drain += 1
                        if n_drain % 2:
                            nc.vector.tensor_copy(out=out, in_=in_)
                        else:
                            nc.scalar.activation(out=out, in_=in_,
                                                 func=AF.Copy, bias=0.0,
                                                 scale=1.0)

                    for g in range(4):
                        qka = p2.tile([65, 64 * 256], BF16, tag="qka",
                                      name=f"qka_{g}")
                        va = p2.tile([64, 64 * 128], BF16, tag="va",
                                     name=f"va_{g}")
                        # qk conv: 2 column-pairs per psum group, PE bias row
                        for spp in range(16):
                            pq = psA.tile([128, 512], F32, tag="conv",
                                          name=f"pq_{g}_{spp}")
                            for q in range(2):
                                sp = 2 * spp + q
                                for kt in range(4):
                                    nc.tensor.matmul(
                                        pq[:, q * 256:(q + 1) * 256],
                                        xT[kt][:, sp * 128:(sp + 1) * 128],
                                        wqkT[kt][:, g * 256:(g + 1) * 256],
                                        start=(q == 0 and kt == 0),
                                        stop=False,
                                        skip_group_check=True)
                            nc.tensor.matmul(
                                pq, ones1,
                                _slice_part(bqkB, 0, 1, g * 256,
                                            [[0, 2], [1, 256]]),
                                start=False, stop=True,
                                skip_group_check=True)
                            for j in range(2):
                                drain(
                                    _slice_part(qka, 0, 64, (4 * spp + j) * 256,
                                                [[512, 2], [1, 256]]),
                                    pq[64 * j:64 * (j + 1), :])
                        # qka 65th row: k-slots = 1.0, q-slots = edge[h,i,w]
                        nc.scalar.dma_start(
                            out=_slice_part(qka, 64, 1, 64,
                                            [[256, 64], [128, 2], [1, 64]]),
                            in_=_slice_part(ones1, 0, 1, 0, [[0, 128], [1, 64]]))
                        for hh in range(2):
                            nc.scalar.dma_start(
                                out=_slice_part(qka, 64, 1, hh * 128,
                                                [[256, 64], [1, 64]]),
                                in_=_slice_part(edge_flT, 2 * g + hh, 1, 0,
                                                [[64, 64], [1, 64]]))
                        # QK^T logits (K=65 incl. edge row), exp from
                        # psum, interleaved with the v conv so the PE hops
                        # to a psA vconv group whenever QK stalls on the
                        # exp-paced psB pool
                        Sx = [p2.tile([64, SP], BF16, tag=f"Sx{hh}",
                                      name=f"Sx_{g}_{hh}")
                              for hh in range(2)]
                        for s in range(16):
                            hh, ib = s // 8, s % 8
                            qoff, koff = hh * 128, hh * 128 + 64
                            sp_ = psB.tile([64, 512], F32, tag="att",
                                           name=f"sp_{g}_{hh}_{ib}")
                            for ii in range(8):
                                i = ib * 8 + ii
                                out_ap = bass.AP(
                                    tensor=sp_.tensor,
                                    offset=sp_.offset + ii,
                                    ap=[sp_.ap[0], [8, 64]])
                                nc.tensor.matmul(
                                    out_ap,
                                    _ap(qka, koff + i, [[256, 64]]),
                                    _ap(qka, qoff + i, [[256, 64]]),
                                    start=(ii == 0), stop=(ii == 7),
                                    skip_group_check=True)
                            nc.scalar.activation(
                                out=_ap(Sx[hh], ib * 8,
                                        [[64, 64], [1, 8]]),
                                in_=sp_, func=AF.Exp)
                            dq = s
                            pv = psA.tile([64, 512], F32, tag="conv",
                                          name=f"pv_{g}_{dq}")
                            for q in range(4):
                                d0 = 4 * dq + q
                                for kt in range(4):
                                    lhs = _ap(x65[kt], d0 * W65 + 1, [[1, 64]])
                                    nc.tensor.matmul(
                                        pv[:, q * 128:(q + 1) * 128], lhs,
                                        wvT[kt][:, g * 128:(g + 1) * 128],
                                        start=(q == 0 and kt == 0),
                                        stop=False,
                                        skip_group_check=True)
                            nc.tensor.matmul(
                                pv, ones1[:, 0:64],
                                _slice_part(bvB, 0, 1, g * 128,
                                            [[0, 4], [1, 128]]),
                                start=False, stop=True,
                                skip_group_check=True)
                            drain(_ap(va, 4 * dq * 128, [[1, 512]]), pv)
                        # softmax over i (contiguous), both heads first
                        for hh in range(2):
                            D = p2.tile([64, 64], F32, tag=f"D{hh}",
                                        name=f"D_{g}_{hh}")
                            Pv = bass.AP(tensor=Sx[hh].tensor,
                                         offset=Sx[hh].offset,
                                         ap=[Sx[hh].ap[0], [64, 64], [1, 64]])
                            nc.vector.reduce_sum(out=D, in_=Pv,
                                                 axis=mybir.AxisListType.X)
                            nc.vector.reciprocal(out=D, in_=D)
                            Rb = bass.AP(tensor=D.tensor, offset=D.offset,
                                         ap=[D.ap[0], [1, 64], [0, 64]])
                            nc.vector.tensor_mul(out=Sx[hh], in0=Sx[hh],
                                                 in1=Rb)
                        # AV per head
                        for hh in range(2):
                            h = 2 * g + hh
                            for ib in range(8):
                                ap2 = psB.tile([64, 512], F32, tag="att",
                                               name=f"ap2_{g}_{hh}_{ib}")
                                for ii in range(8):
                                    i = ib * 8 + ii
                                    nc.tensor.matmul(
                                        ap2[:, ii * 64:(ii + 1) * 64],
                                        _ap(va, hh * 64 + i, [[128, 64]]),
                                        _ap(Sx[hh], i, [[64, 64]]),
                                        start=(ii == 0), stop=(ii == 7),
                                        skip_group_check=True)
                                blk = glob.tile([64, 512], BF16, tag="blk",
                                                bufs=8,
                                                name=f"blk_{g}_{hh}_{ib}")
                                drain(blk, ap2)
                                for ii in range(8):
                                    i = ib * 8 + ii
                                    c = h * 64 + i
                                    kt, p = c // 128, c % 128
                                    rot = (nc.gpsimd, nc.sync, nc.gpsimd)
                                    eng = rot[n_agg_dma % 3]
                                    n_agg_dma += 1
                                    eng.dma_start(
                                        out=_slice_part(
                                            agg65[kt], p, 1, 1,
                                            [[W65, 64], [1, 64]]),
                                        in_=blk[:, ii * 64:(ii + 1) * 64])

            # ================= Phases 3-4 pool =================
            with tc.tile_pool(name="tail", bufs=1) as tail:
                yt = [tail.tile([128, SP], F32, name=f"y_{k}") for k in range(4)]
                stats_l = tail.tile([128, 8], F32)
                # pass-2 weights preload on sync at tail-open; pass-1's
                # rotating pool nests inside so their spaces are disjoint
                with tc.tile_pool(name="p3w2", bufs=1) as p3w2:
                    fwa = {}
                    for ct in range(4):
                        for t_i in range(9):
                            for kt in range(4):
                                w = p3w2.tile([128, 128], BF16,
                                              name=f"fwa_{ct}_{t_i}_{kt}")
                                nc.sync.dma_start(
                                    out=w, in_=d_fwT[t_i, kt + 4, ct, :, :])
                                fwa[(ct, t_i, kt)] = w
                    # pass 1: x half of the fusion conv
                    with tc.tile_pool(name="p3w", bufs=2) as p3w:
                        for ct in range(4):
                            if ct == 0:
                                fw = fw0
                            else:
                                fw = {}
                                for t_i in range(9):
                                    for kt in range(4):
                                        w = p3w.tile(
                                            [128, 128], BF16,
                                            tag=f"fwx_{t_i}_{kt}",
                                            name=f"fwx_{ct}_{t_i}_{kt}")
                                        nc.scalar.dma_start(
                                            out=w,
                                            in_=d_fwT[t_i, kt, ct, :, :])
                                        fw[(t_i, kt)] = w
                            for ch in range(8):
                                pf = psA.tile([128, 512], F32, tag="conv",
                                              name=f"pf_{ct}_{ch}")
                                conv3x3_chunk(nc, pf, fw, x65, ch * 8, 4)
                                nc.scalar.activation(
                                    out=yt[ct][:, ch * 512:(ch + 1) * 512],
                                    in_=pf, func=AF.Identity,
                                    bias=fb[:, ct:ct + 1], scale=1.0)
                    # pass 2: agg half, accumulated into yt on DVE
                    for ct in range(4):
                        fw = {(t_i, kt): fwa[(ct, t_i, kt)]
                              for t_i in range(9) for kt in range(4)}
                        for ch in range(8):
                            pf = psA.tile([128, 512], F32, tag="conv",
                                          name=f"pf2_{ct}_{ch}")
                            conv3x3_chunk(nc, pf, fw, agg65, ch * 8, 4)
                            nc.vector.tensor_add(
                                out=yt[ct][:, ch * 512:(ch + 1) * 512],
                                in0=yt[ct][:, ch * 512:(ch + 1) * 512],
                                in1=pf)
                        # per-ct BN partial stats, overlapped with next ct
                        st = tail.tile([128, 8, 6], F32, tag="st",
                                       name=f"st_{ct}")
                        for j in range(8):
                            nc.vector.bn_stats(
                                out=st[:, j, :],
                                in_=yt[ct][:, j * 512:(j + 1) * 512])
                        mv4 = tail.tile([128, 2], F32, tag="mv4",
                                        name=f"mv4_{ct}")
                        nc.vector.bn_aggr(out=mv4, in_=st)
                        nc.scalar.activation(out=stats_l[:, 2 * ct:2 * ct + 1],
                                             in_=mv4[:, 0:1], func=AF.Copy,
                                             bias=0.0, scale=float(SP))
                        sq = tail.tile([128, 1], F32, tag="sq", name=f"sq_{ct}")
                        nc.vector.tensor_mul(out=sq, in0=mv4[:, 0:1],
                                             in1=mv4[:, 0:1])
                        nc.vector.tensor_add(out=sq, in0=sq, in1=mv4[:, 1:2])
                        nc.scalar.activation(out=stats_l[:, 2 * ct + 1:2 * ct + 2],
                                             in_=sq, func=AF.Copy,
                                             bias=0.0, scale=float(SP))

                with tc.tile_pool(name="p4", bufs=1) as p4:
                    bng = p4.tile([128, 4], F32)
                    nc.sync.dma_start(
                        out=bng, in_=d_bng[:].rearrange("(a b) c -> b (a c)", a=4))
                    bnb = p4.tile([128, 4], F32)
                    nc.sync.dma_start(
                        out=bnb, in_=d_bnb[:].rearrange("(a b) c -> b (a c)", a=4))
                    cc_in = dram.tile([128, 8], F32)
                    cc_out = dram.tile([128, 8], F32)
                    nc.gpsimd.dma_start(out=cc_in, in_=stats_l)
                    nc.gpsimd.collective_compute(
                        "AllReduce", mybir.AluOpType.add,
                        replica_groups=[list(range(N_CORES))],
                        ins=[cc_in.opt()], outs=[cc_out.opt()])
                    rstats = p4.tile([128, 8], F32)
                    nc.sync.dma_start(out=rstats, in_=cc_out)
                    eps128 = p4.tile([128, 1], F32)
                    nc.vector.memset(eps128, float(EPS))
                    NTOT = float(B * SP)
                    for ct in range(4):
                        mean = p4.tile([128, 1], F32, tag="mean", name=f"mn_{ct}")
                        nc.scalar.activation(out=mean,
                                             in_=rstats[:, 2 * ct:2 * ct + 1],
                                             func=AF.Copy, bias=0.0,
                                             scale=1.0 / NTOT)
                        var = p4.tile([128, 1], F32, tag="var", name=f"vr_{ct}")
                        nc.vector.tensor_mul(out=var, in0=mean, in1=mean)
                        ex2t = p4.tile([128, 1], F32, tag="ex2t", name=f"e2_{ct}")
                        nc.scalar.activation(out=ex2t,
                                             in_=rstats[:, 2 * ct + 1:2 * ct + 2],
                                             func=AF.Copy, bias=0.0,
                                             scale=1.0 / NTOT)
                        nc.vector.tensor_sub(out=var, in0=ex2t, in1=var)
                        nc.scalar.activation(out=var, in_=var, func=AF.Sqrt,
                                             bias=eps128, scale=1.0)
                        nc.vector.reciprocal(out=var, in_=var)
                        sc = p4.tile([128, 1], F32, tag="sc", name=f"sc_{ct}")
                        nc.vector.tensor_mul(out=sc, in0=var,
                                             in1=bng[:, ct:ct + 1])
                        sh = p4.tile([128, 1], F32, tag="sh", name=f"sh_{ct}")
                        nc.vector.tensor_mul(out=sh, in0=mean, in1=sc)
                        nc.vector.tensor_sub(out=sh, in0=bnb[:, ct:ct + 1],
                                             in1=sh)
                        sg = p4.tile([128, SP], F32, tag="sg", bufs=2,
                                     name=f"sg_{ct}")
                        nc.scalar.activation(out=sg, in_=yt[ct], func=AF.Silu,
                                             bias=sh, scale=sc)
                        oeng = (nc.sync, nc.gpsimd, nc.scalar, nc.sync)[ct]
                        oeng.dma_start(out=d_y[ct * 128:(ct + 1) * 128, :],
                                       in_=sg)

    _split_multi_waits(nc)
    return nc


_PROGRAM = None


def _get_program():
    global _PROGRAM
    if _PROGRAM is None:
        _PROGRAM = build_program()
    return _PROGRAM


def _bf16(a):
    return np.ascontiguousarray(np.asarray(a, np.float32).astype(ml_dtypes.bfloat16))


def _f32(a):
    return np.ascontiguousarray(np.asarray(a, np.float32))


def kernel(x, box_w1, box_b1, box_w2, box_b2, edge_w1, edge_b1, gn_g, gn_b,
           edge_w2, edge_b2, qkv_w, qkv_b, fus_w, fus_b, bn_g, bn_b,
           trace=False):
    global LAST_RESULTS
    x = np.asarray(x, np.float32)
    scale = float(HD) ** -0.5

    qkv_w2 = np.asarray(qkv_w, np.float32).reshape(3 * C, C)
    qkv_b2 = np.asarray(qkv_b, np.float32).copy()
    wq = qkv_w2[0:C] * scale
    bq = qkv_b2[0:C] * scale
    wk, bk = qkv_w2[C:2 * C], qkv_b2[C:2 * C]
    wv, bv_ = qkv_w2[2 * C:], qkv_b2[2 * C:]
    wqk = np.empty((1024, C), np.float32)
    bqk = np.empty(1024, np.float32)
    for h in range(NH):
        wqk[h * 128:h * 128 + 64] = wq[h * 64:(h + 1) * 64]
        wqk[h * 128 + 64:(h + 1) * 128] = wk[h * 64:(h + 1) * 64]
        bqk[h * 128:h * 128 + 64] = bq[h * 64:(h + 1) * 64]
        bqk[h * 128 + 64:(h + 1) * 128] = bk[h * 64:(h + 1) * 64]

    bw1T = np.asarray(box_w1, np.float32).transpose(2, 3, 1, 0).reshape(9, C, 64)
    ew1T = np.asarray(edge_w1, np.float32).transpose(2, 3, 1, 0).reshape(9, 4, 64)
    fwT = np.asarray(fus_w, np.float32).transpose(2, 3, 1, 0).reshape(9, 1024, C)
    fwT_t = np.ascontiguousarray(
        fwT.reshape(9, 8, 128, 4, 128).transpose(0, 1, 3, 2, 4))

    gmat = np.zeros((64, 8), np.float32)
    for g in range(8):
        gmat[g * 8:(g + 1) * 8, g] = 1.0

    shared = {
        "wqkT": _bf16(wqk.T), "wvT": _bf16(wv.T),
        "bqkB": _bf16(bqk[None, :]),
        "bvB": _bf16(bv_[None, :]),
        "bw1": _bf16(bw1T), "bb1": _f32(np.asarray(box_b1).reshape(64, 1)),
        "bw2": _bf16(np.asarray(box_w2, np.float32).reshape(4, 64).T),
        "bb2": _f32(np.asarray(box_b2).reshape(4, 1)),
        "ew1": _bf16(ew1T), "eb1": _f32(np.asarray(edge_b1).reshape(64, 1)),
        "ew2": _bf16(np.asarray(edge_w2, np.float32).reshape(8, 64).T),
        "eb2": _f32(np.asarray(edge_b2).reshape(8, 1)),
        "gng": _f32(np.asarray(gn_g).reshape(64, 1)),
        "gnb": _f32(np.asarray(gn_b).reshape(64, 1)),
        "gmat": gmat,
        "fwT": _bf16(fwT_t),
        "fb": _f32(np.asarray(fus_b).reshape(C, 1)),
        "bng": _f32(np.asarray(bn_g).reshape(C, 1)),
        "bnb": _f32(np.asarray(bn_b).reshape(C, 1)),
    }

    in_maps = []
    for b in range(B):
        xb = x[b]
        x65h = np.zeros((C, 64, W65), np.float32)
        x65h[:, :, 1:] = xb
        x65h = np.concatenate(
            [x65h.reshape(C, 4160), np.zeros((C, 4), np.float32)], axis=1)
        m = dict(shared)
        m["x65"] = _bf16(x65h)
        m["xT"] = _bf16(np.ascontiguousarray(xb.transpose(0, 2, 1)).reshape(C, SP))
        in_maps.append(m)

    nc = _get_program()
    res = run_bass_kernel_spmd(nc, in_maps, core_ids=list(range(N_CORES)),
                               trace=trace)
    LAST_RESULTS = res
    out = np.empty((B, C, H, W), np.float32)
    for b in range(B):
        out[b] = res.results[b]["y"].reshape(C, H, W)
    return out


# revision 51
# speedup vs baseline: 1.0906x; 1.0906x over previous
"""AutoBoxGraphAttention Trainium2 kernel (optimized).

Data-parallel over batch: core b handles image b (B=8, one per NeuronCore).
The only cross-core communication is a 4KB AllReduce of BatchNorm partials.

Key layout/scheduling choices vs the naive version:
  - qkv biases folded into the PSUM->SBUF copies (tensor_tensor add with
    host-precomputed partition-broadcast bias tiles). No K=1 bias matmuls.
  - edge bias folded into the QK^T contraction as a 65th K row: qka has a
    65th partition whose q-slots hold edge[h,i,w] and k-slots hold 1.0.
  - v conv processes two image rows per matmul (N=128, full PE width).
  - S uses transposed free layout S[W', w*64+i] so the softmax reduction
    over i is contiguous; exp() reads PSUM directly (no S copy).
  - agg returns to conv world via SBUF->SBUF DMAs (no DRAM bounce, no
    readback), issue alternating sync/gpsimd queues.
  - fusion conv runs as two passes: the x half (independent of attention)
    right after attention to hide the agg DMA drain, then the agg half
    accumulated into yt with DVE adds.

Spatial layout "w65": each row padded to 65 elements with a LEADING zero
(index d*65 holds 0, data at d*65+1 .. +64, plus 4 trailing zeros; total
4164). A 3x3 tap (dy,dx) then reads a plain strided AP at offset
(r+dy-1)*65 + dx; horizontal SAME-padding is automatic, vertical padding
via per-tap row clamping.
"""

import os
import sys

for _p in ("/opt/trn_rl_repo", os.path.dirname(os.path.abspath(__file__))):
    if _p not in sys.path:
        sys.path.insert(0, _p)

import numpy as np
import ml_dtypes

import concourse.tile as _tile_mod


def _apply_toolchain_patches():
    """This container's walrus accepts at most ONE sync-wait per
    instruction; Tile's exit drain and scheduler attach several. Split the
    exit drain into single-wait drains, and post-process the module to
    hoist extra waits onto same-engine NoOps."""
    import concourse.mybir as mybir

    def _split_drain_and_barrier(self, tick_clock, wait_clock):
        from concourse.tile import ScopedClock
        nc = self.nc
        drain_inst = nc.sync.drain()
        wait_clock.add_sem_waits(
            drain_inst.ins, ScopedClock({None: tick_clock.global_clock}))
        si = drain_inst.ins.sync_info
        if si is not None and len(si.on_wait) > 1:
            waits = list(si.on_wait)
            drain_inst.ins.sync_info = type(si)(
                on_wait=waits[:1], on_update=list(si.on_update))
            for w in waits[1:]:
                d2 = nc.sync.drain()
                si2 = d2.ins.sync_info
                if si2 is None:
                    d2.ins.sync_info = type(si)(on_wait=[w], on_update=[])
                else:
                    d2.ins.sync_info = type(si2)(
                        on_wait=list(si2.on_wait) + [w],
                        on_update=list(si2.on_update))
        nc.all_engine_barrier()
        assert self.sems is not None
        popped = nc._tile_sem_poison_stack.pop()
        assert popped is self._sem_poison
        nc.clear_and_free_semaphores(list(self.sems.allocated().values()))
        nc.all_engine_barrier()

    _tile_mod.TileContext._drain_and_barrier = _split_drain_and_barrier


def _split_multi_waits(nc):
    import concourse.mybir as mybir
    n_split = 0
    for fn in nc.m.functions:
        for bb in fn.blocks:
            insts = list(bb.instructions)
            out = []
            changed = False
            for inst in insts:
                si = inst.sync_info
                if si is not None and len(si.on_wait) > 1:
                    waits = list(si.on_wait)
                    for w in waits[:-1]:
                        nop = mybir.InstNoOp(
                            name=f"{inst.name}-wsplit{n_split}",
                            engine=inst.engine, bass_nofuse=True)
                        nop.sync_info = mybir.SyncInfo(on_wait=[w], on_update=[])
                        out.append(nop)
                        n_split += 1
                    inst.sync_info = type(si)(
                        on_wait=[waits[-1]], on_update=list(si.on_update))
                    changed = True
                out.append(inst)
            if changed:
                bb.instructions = out
    return n_split


_apply_toolchain_patches()

import concourse.bass as bass  # noqa: E402
import concourse.tile as tile  # noqa: E402
from concourse import mybir  # noqa: E402
from concourse.bass_utils import run_bass_kernel_spmd  # noqa: E402

F32 = mybir.dt.float32
BF16 = mybir.dt.bfloat16

B, C, H, W = 8, 512, 64, 64
NH, HD = 8, 64
EPS = 1e-5
W65 = 65
SP65 = 64 * W65 + 4  # 4164
SP = 4096
N_CORES = 8

LAST_RESULTS = None


def _ap(t, offset, dims):
    return bass.AP(tensor=t.tensor, offset=t.offset + offset, ap=[t.ap[0]] + dims)


def _slice_part(t, p0, np_, offset, dims):
    # tile[p0:p0+np_] then rebuild free dims
    sub = t[p0:p0 + np_]
    return bass.AP(tensor=sub.tensor, offset=sub.offset + offset,
                   ap=[sub.ap[0]] + dims)


TAPS = [(1, 0), (1, 1), (1, 2), (0, 0), (0, 1), (0, 2), (2, 0), (2, 1), (2, 2)]


def conv3x3_chunk(nc, psum, w_tiles, src_tiles, r0, n_ktiles, co=128):
    """3x3 conv, one 8-row chunk, accumulated into psum (co, 8*64).
    w_tiles[(tap_idx, kt)] = lhsT (K, co); src_tiles[kt] = w65 tile.
    dy=1 taps first (full coverage -> start=True clears the bank)."""
    mms = []
    for dy, dx in TAPS:
        t_i = dy * 3 + dx  # host weight tap order is (ky, kx) row-major
        a = max(r0, 1 - dy)
        b = min(r0 + 8, 65 - dy)
        nr = b - a
        if nr <= 0:
            continue
        for kt in range(n_ktiles):
            mms.append((t_i, kt, a, nr, dy, dx))
    for j, (t_i, kt, a, nr, dy, dx) in enumerate(mms):
        src = src_tiles[kt]
        in_ap = _ap(src, (a + dy - 1) * W65 + dx, [[W65, nr], [1, 64]])
        nc.tensor.matmul(
            psum[0:co, (a - r0) * 64:(a - r0 + nr) * 64], w_tiles[(t_i, kt)],
            in_ap, start=(j == 0), stop=(j == len(mms) - 1),
            skip_group_check=True)


def build_program():
    nc = bass.Bass(trn_type="TRN2", num_devices=N_CORES)

    d_x65 = nc.dram_tensor("x65", [C, SP65], BF16, kind="ExternalInput")
    d_xT = nc.dram_tensor("xT", [C, SP], BF16, kind="ExternalInput")
    d_wqkT = nc.dram_tensor("wqkT", [C, 1024], BF16, kind="ExternalInput")
    d_wvT = nc.dram_tensor("wvT", [C, 512], BF16, kind="ExternalInput")
    d_bqkB = nc.dram_tensor("bqkB", [1, 1024], BF16, kind="ExternalInput")
    d_bvB = nc.dram_tensor("bvB", [1, 512], BF16, kind="ExternalInput")
    d_bw1 = nc.dram_tensor("bw1", [9, C, 64], BF16, kind="ExternalInput")
    d_bb1 = nc.dram_tensor("bb1", [64, 1], F32, kind="ExternalInput")
    d_bw2 = nc.dram_tensor("bw2", [64, 4], BF16, kind="ExternalInput")
    d_bb2 = nc.dram_tensor("bb2", [4, 1], F32, kind="ExternalInput")
    d_ew1 = nc.dram_tensor("ew1", [9, 4, 64], BF16, kind="ExternalInput")
    d_eb1 = nc.dram_tensor("eb1", [64, 1], F32, kind="ExternalInput")
    d_ew2 = nc.dram_tensor("ew2", [64, 8], BF16, kind="ExternalInput")
    d_eb2 = nc.dram_tensor("eb2", [8, 1], F32, kind="ExternalInput")
    d_gng = nc.dram_tensor("gng", [64, 1], F32, kind="ExternalInput")
    d_gnb = nc.dram_tensor("gnb", [64, 1], F32, kind="ExternalInput")
    d_gmat = nc.dram_tensor("gmat", [64, 8], F32, kind="ExternalInput")
    d_fwT = nc.dram_tensor("fwT", [9, 8, 4, 128, 128], BF16, kind="ExternalInput")
    d_fb = nc.dram_tensor("fb", [C, 1], F32, kind="ExternalInput")
    d_bng = nc.dram_tensor("bng", [C, 1], F32, kind="ExternalInput")
    d_bnb = nc.dram_tensor("bnb", [C, 1], F32, kind="ExternalInput")
    d_y = nc.dram_tensor("y", [C, SP], F32, kind="ExternalOutput")

    AF = mybir.ActivationFunctionType

    with tile.TileContext(nc) as tc:
        with tc.tile_pool(name="glob", bufs=1) as glob, \
             tc.tile_pool(name="psA", bufs=4, space="PSUM") as psA, \
             tc.tile_pool(name="psB", bufs=4, space="PSUM") as psB, \
             tc.tile_pool(name="dram", bufs=1, space="DRAM") as dram:

            x65 = [glob.tile([128, SP65], BF16, name=f"x65_{k}")
                   for k in range(4)]
            agg65 = [glob.tile([128, SP65], BF16, name=f"agg65_{k}")
                     for k in range(4)]
            ones1 = glob.tile([1, 128], BF16)
            fb = glob.tile([128, 4], F32)
            nc.sync.dma_start(
                out=fb, in_=d_fb[:].rearrange("(a b) c -> b (a c)", a=4))
            for k in range(4):
                nc.sync.dma_start(out=x65[k], in_=d_x65[k * 128:(k + 1) * 128, :])
                nc.gpsimd.memset(agg65[k], 0.0)
            nc.vector.memset(ones1, 1.0)
            fw0 = {}
            for t_i in range(9):
                for kt in range(4):
                    fw0[(t_i, kt)] = glob.tile([128, 128], BF16,
                                               name=f"fw0_{t_i}_{kt}")

            # ================= Phases 1-2 pool =================
            with tc.tile_pool(name="ph12", bufs=1) as ph12:
                xT = [ph12.tile([128, SP], BF16, name=f"xT_{k}")
                      for k in range(4)]
                wqkT = [ph12.tile([128, 1024], BF16, name=f"wqkT_{kt}")
                        for kt in range(4)]
                wvT = [ph12.tile([128, 512], BF16, name=f"wvT_{kt}")
                       for kt in range(4)]
                for kt in range(4):
                    nc.gpsimd.dma_start(out=wqkT[kt],
                                        in_=d_wqkT[kt * 128:(kt + 1) * 128, :])
                    nc.gpsimd.dma_start(out=wvT[kt],
                                        in_=d_wvT[kt * 128:(kt + 1) * 128, :])
                bqkB = ph12.tile([1, 1024], BF16)
                nc.gpsimd.dma_start(out=bqkB, in_=d_bqkB[:])
                bvB = ph12.tile([1, 512], BF16)
                nc.gpsimd.dma_start(out=bvB, in_=d_bvB[:])
                edge_flT = ph12.tile([8, SP], BF16)

                # ---------- Phase 1: box_net + edge_net ----------
                with tc.tile_pool(name="p1", bufs=1) as p1:
                    for k in range(4):
                        nc.gpsimd.dma_start(out=xT[k],
                                            in_=d_xT[k * 128:(k + 1) * 128, :])
                    bw1 = {}
                    for t_i in range(9):
                        for kt in range(4):
                            w = p1.tile([128, 64], BF16, name=f"bw1_{t_i}_{kt}")
                            nc.sync.dma_start(
                                out=w, in_=d_bw1[t_i, kt * 128:(kt + 1) * 128, :])
                            bw1[(t_i, kt)] = w
                    bb1 = p1.tile([64, 1], F32)
                    nc.sync.dma_start(out=bb1, in_=d_bb1[:])
                    bw2 = p1.tile([64, 4], BF16)
                    nc.sync.dma_start(out=bw2, in_=d_bw2[:])
                    bb2 = p1.tile([4, 1], F32)
                    nc.sync.dma_start(out=bb2, in_=d_bb2[:])
                    ew1 = {}
                    for t_i in range(9):
                        w = p1.tile([4, 64], BF16, name=f"ew1_{t_i}")
                        nc.sync.dma_start(out=w, in_=d_ew1[t_i, :, :])
                        ew1[(t_i, 0)] = w
                    eb1 = p1.tile([64, 1], F32)
                    nc.sync.dma_start(out=eb1, in_=d_eb1[:])
                    ew2 = p1.tile([64, 8], BF16)
                    nc.sync.dma_start(out=ew2, in_=d_ew2[:])
                    eb2 = p1.tile([8, 1], F32)
                    nc.sync.dma_start(out=eb2, in_=d_eb2[:])
                    gng = p1.tile([64, 1], F32)
                    nc.sync.dma_start(out=gng, in_=d_gng[:])
                    gnb = p1.tile([64, 1], F32)
                    nc.sync.dma_start(out=gnb, in_=d_gnb[:])
                    gmat = p1.tile([64, 8], F32)
                    nc.sync.dma_start(out=gmat, in_=d_gmat[:])
                    for (t_i, kt), w in fw0.items():
                        nc.sync.dma_start(out=w, in_=d_fwT[t_i, kt, 0, :, :])

                    box1 = p1.tile([64, SP65], BF16)
                    nc.vector.memset(box1, 0.0)
                    for ch in range(8):
                        pb = psA.tile([128, 512], F32, tag="conv", name=f"pb_{ch}")
                        conv3x3_chunk(nc, pb, bw1, x65, ch * 8, 4, co=64)
                        nc.scalar.activation(
                            out=_slice_part(box1, 0, 64, ch * 8 * W65 + 1,
                                            [[W65, 8], [1, 64]]),
                            in_=pb[0:64, :], func=AF.Gelu, bias=bb1, scale=1.0)

                    boxes = p1.tile([4, SP65], BF16)
                    nc.vector.memset(boxes, 0.0)
                    for ch in range(8):
                        pb2 = psA.tile([128, 512], F32, tag="conv", name=f"pb2_{ch}")
                        nc.tensor.matmul(
                            pb2[0:4, :], bw2,
                            _ap(box1, ch * 8 * W65 + 1, [[W65, 8], [1, 64]]),
                            start=True, stop=True)
                        nc.scalar.activation(
                            out=_slice_part(boxes, 0, 4, ch * 8 * W65 + 1,
                                            [[W65, 8], [1, 64]]),
                            in_=pb2[0:4, :], func=AF.Sigmoid, bias=bb2, scale=1.0)

                    e1 = p1.tile([64, SP], F32)
                    for ch in range(8):
                        pe = psA.tile([128, 512], F32, tag="conv", name=f"pe_{ch}")
                        conv3x3_chunk(nc, pe, ew1, [boxes], ch * 8, 1, co=64)
                        nc.scalar.activation(
                            out=e1[:, ch * 512:(ch + 1) * 512], in_=pe[0:64, :],
                            func=AF.Identity, bias=eb1, scale=1.0)

                    stats = p1.tile([64, 8, 6], F32)
                    for j in range(8):
                        nc.vector.bn_stats(out=stats[:, j, :],
                                           in_=e1[:, j * 512:(j + 1) * 512])
                    mv = p1.tile([64, 2], F32)
                    nc.vector.bn_aggr(out=mv, in_=stats)
                    ex2 = p1.tile([64, 2], F32)
                    nc.vector.tensor_copy(out=ex2[:, 0:1], in_=mv[:, 0:1])
                    nc.vector.tensor_mul(out=ex2[:, 1:2], in0=mv[:, 0:1],
                                         in1=mv[:, 0:1])
                    nc.vector.tensor_add(out=ex2[:, 1:2], in0=ex2[:, 1:2],
                                         in1=mv[:, 1:2])
                    gs_ps = psB.tile([8, 2], F32, tag="att", name="gs_ps")
                    nc.tensor.matmul(gs_ps, gmat, ex2, start=True, stop=True)
                    gs = p1.tile([8, 2], F32)
                    nc.scalar.activation(out=gs, in_=gs_ps, func=AF.Copy,
                                         bias=0.0, scale=1.0 / 8.0)
                    gvar = p1.tile([8, 1], F32)
                    eps8 = p1.tile([8, 1], F32)
                    nc.vector.memset(eps8, float(EPS))
                    nc.vector.tensor_mul(out=gvar, in0=gs[:, 0:1], in1=gs[:, 0:1])
                    nc.vector.tensor_sub(out=gvar, in0=gs[:, 1:2], in1=gvar)
                    nc.scalar.activation(out=gvar, in_=gvar, func=AF.Sqrt,
                                         bias=eps8, scale=1.0)
                    nc.vector.reciprocal(out=gvar, in_=gvar)
                    gmr = p1.tile([8, 2], F32)
                    nc.vector.tensor_copy(out=gmr[:, 0:1], in_=gs[:, 0:1])
                    nc.vector.tensor_copy(out=gmr[:, 1:2], in_=gvar)
                    cmr = p1.tile([64, 2], F32)
                    src_bc = bass.AP(tensor=gmr.tensor, offset=gmr.offset,
                                     ap=[[gmr.ap[0][0], 8], [0, 8], [1, 2]])
                    nc.sync.dma_start(out=cmr, in_=src_bc)
                    gsc = p1.tile([64, 1], F32)
                    nc.vector.tensor_mul(out=gsc, in0=cmr[:, 1:2], in1=gng)
                    gsh = p1.tile([64, 1], F32)
                    nc.vector.tensor_mul(out=gsh, in0=cmr[:, 0:1], in1=gsc)
                    nc.vector.tensor_sub(out=gsh, in0=gnb, in1=gsh)
                    e1g = p1.tile([64, SP], BF16)
                    nc.scalar.activation(out=e1g, in_=e1, func=AF.Gelu,
                                         bias=gsh, scale=gsc)
                    # edge conv2 with transposed moving AP -> edge_flT[h, w*64+i]
                    for wb in range(8):
                        pe2 = psA.tile([128, 512], F32, tag="conv",
                                       name=f"pe2_{wb}")
                        nc.tensor.matmul(
                            pe2[0:8, :], ew2,
                            _ap(e1g, wb * 8, [[1, 8], [64, 64]]),
                            start=True, stop=True)
                        nc.scalar.activation(
                            out=edge_flT[:, wb * 512:(wb + 1) * 512],
                            in_=pe2[0:8, :], func=AF.Identity, bias=eb2,
                            scale=1.0)

                # ---------- Phase 2: qkv conv + attention ----------
                with tc.tile_pool(name="p2", bufs=1) as p2:
                    n_agg_dma = 0
                    n_drain = 0

                    def drain(out, in_):
                        nonlocal n_drain
                        n_drain += 1
                        if n_drain % 2:
                            nc.vector.tensor_copy(out=out, in_=in_)
                        else:
                            nc.scalar.activation(out=out, in_=in_,
                                                 func=AF.Copy, bias=0.0,
                                                 scale=1.0)

                    for g in range(4):
                        qka = p2.tile([65, 64 * 256], BF16, tag="qka",
                                      name=f"qka_{g}")
                        va = p2.tile([64, 64 * 128], BF16, tag="va",
                                     name=f"va_{g}")
                        # qk conv: 2 column-pairs per psum group, PE bias row
                        for spp in range(16):
                            pq = psA.tile([128, 512], F32, tag="conv",
                                          name=f"pq_{g}_{spp}")
                            for q in range(2):
                                sp = 2 * spp + q
                                for kt in range(4):
                                    nc.tensor.matmul(
                                        pq[:, q * 256:(q + 1) * 256],
                                        xT[kt][:, sp * 128:(sp + 1) * 128],
                                        wqkT[kt][:, g * 256:(g + 1) * 256],
                                        start=(q == 0 and kt == 0),
                                        stop=False,
                                        skip_group_check=True)
                            nc.tensor.matmul(
                                pq, ones1,
                                _slice_part(bqkB, 0, 1, g * 256,
                                            [[0, 2], [1, 256]]),
                                start=False, stop=True,
                                skip_group_check=True)
                            for j in range(2):
                                drain(
                                    _slice_part(qka, 0, 64, (4 * spp + j) * 256,
                                                [[512, 2], [1, 256]]),
                                    pq[64 * j:64 * (j + 1), :])
                        # qka 65th row: k-slots = 1.0, q-slots = edge[h,i,w]
                        nc.scalar.dma_start(
                            out=_slice_part(qka, 64, 1, 64,
                                            [[256, 64], [128, 2], [1, 64]]),
                            in_=_slice_part(ones1, 0, 1, 0, [[0, 128], [1, 64]]))
                        for hh in range(2):
                            nc.scalar.dma_start(
                                out=_slice_part(qka, 64, 1, hh * 128,
                                                [[256, 64], [1, 64]]),
                                in_=_slice_part(edge_flT, 2 * g + hh, 1, 0,
                                                [[64, 64], [1, 64]]))
                        # QK^T logits (K=65 incl. edge row), exp from psum
                        Sx = [None, None]
                        for hh in range(2):
                            qoff, koff = hh * 128, hh * 128 + 64
                            Sx[hh] = p2.tile([64, SP], BF16, tag=f"Sx{hh}",
                                             name=f"Sx_{g}_{hh}")
                            for ib in range(8):
                                sp_ = psB.tile([64, 512], F32, tag="att",
                                               name=f"sp_{g}_{hh}_{ib}")
                                for ii in range(8):
                                    i = ib * 8 + ii
                                    out_ap = bass.AP(
                                        tensor=sp_.tensor,
                                        offset=sp_.offset + ii,
                                        ap=[sp_.ap[0], [8, 64]])
                                    nc.tensor.matmul(
                                        out_ap,
                                        _ap(qka, koff + i, [[256, 64]]),
                                        _ap(qka, qoff + i, [[256, 64]]),
                                        start=(ii == 0), stop=(ii == 7),
                                        skip_group_check=True)
                                nc.scalar.activation(
                                    out=_ap(Sx[hh], ib * 8,
                                            [[64, 64], [1, 8]]),
                                    in_=sp_, func=AF.Exp)
                        # v conv: 4 rows per psum group (x65 rows
                        # stationary); fills the PE while exp/softmax run
                        for dq in range(16):
                            pv = psA.tile([64, 512], F32, tag="conv",
                                          name=f"pv_{g}_{dq}")
                            for q in range(4):
                                d0 = 4 * dq + q
                                for kt in range(4):
                                    lhs = _ap(x65[kt], d0 * W65 + 1, [[1, 64]])
                                    nc.tensor.matmul(
                                        pv[:, q * 128:(q + 1) * 128], lhs,
                                        wvT[kt][:, g * 128:(g + 1) * 128],
                                        start=(q == 0 and kt == 0),
                                        stop=False,
                                        skip_group_check=True)
                            nc.tensor.matmul(
                                pv, ones1[:, 0:64],
                                _slice_part(bvB, 0, 1, g * 128,
                                            [[0, 4], [1, 128]]),
                                start=False, stop=True,
                                skip_group_check=True)
                            drain(_ap(va, 4 * dq * 128, [[1, 512]]), pv)
                        # softmax over i (contiguous), both heads first
                        for hh in range(2):
                            D = p2.tile([64, 64], F32, tag=f"D{hh}",
                                        name=f"D_{g}_{hh}")
                            Pv = bass.AP(tensor=Sx[hh].tensor,
                                         offset=Sx[hh].offset,
                                         ap=[Sx[hh].ap[0], [64, 64], [1, 64]])
                            nc.vector.reduce_sum(out=D, in_=Pv,
                                                 axis=mybir.AxisListType.X)
                            nc.vector.reciprocal(out=D, in_=D)
                            Rb = bass.AP(tensor=D.tensor, offset=D.offset,
                                         ap=[D.ap[0], [1, 64], [0, 64]])
                            nc.vector.tensor_mul(out=Sx[hh], in0=Sx[hh],
                                                 in1=Rb)
                        # AV per head
                        for hh in range(2):
                            h = 2 * g + hh
                            for ib in range(8):
                                ap2 = psB.tile([64, 512], F32, tag="att",
                                               name=f"ap2_{g}_{hh}_{ib}")
                                for ii in range(8):
                                    i = ib * 8 + ii
                                    nc.tensor.matmul(
                                        ap2[:, ii * 64:(ii + 1) * 64],
                                        _ap(va, hh * 64 + i, [[128, 64]]),
                                        _ap(Sx[hh], i, [[64, 64]]),
                                        start=(ii == 0), stop=(ii == 7),
                                        skip_group_check=True)
                                blk = glob.tile([64, 512], BF16, tag="blk",
                                                bufs=8,
                                                name=f"blk_{g}_{hh}_{ib}")
                                drain(blk, ap2)
                                for ii in range(8):
                                    i = ib * 8 + ii
                                    c = h * 64 + i
                                    kt, p = c // 128, c % 128
                                    rot = (nc.gpsimd, nc.sync, nc.gpsimd)
                                    eng = rot[n_agg_dma % 3]
                                    n_agg_dma += 1
                                    eng.dma_start(
                                        out=_slice_part(
                                            agg65[kt], p, 1, 1,
                                            [[W65, 64], [1, 64]]),
                                        in_=blk[:, ii * 64:(ii + 1) * 64])

            # ================= Phases 3-4 pool =================
            with tc.tile_pool(name="tail", bufs=1) as tail:
                yt = [tail.tile([128, SP], F32, name=f"y_{k}") for k in range(4)]
                stats_l = tail.tile([128, 8], F32)
                # pass-2 weights preload on sync at tail-open; pass-1's
                # rotating pool nests inside so their spaces are disjoint
                with tc.tile_pool(name="p3w2", bufs=1) as p3w2:
                    fwa = {}
                    for ct in range(4):
                        for t_i in range(9):
                            for kt in range(4):
                                w = p3w2.tile([128, 128], BF16,
                                              name=f"fwa_{ct}_{t_i}_{kt}")
                                nc.sync.dma_start(
                                    out=w, in_=d_fwT[t_i, kt + 4, ct, :, :])
                                fwa[(ct, t_i, kt)] = w
                    # pass 1: x half of the fusion conv
                    with tc.tile_pool(name="p3w", bufs=2) as p3w:
                        for ct in range(4):
                            if ct == 0:
                                fw = fw0
                            else:
                                fw = {}
                                for t_i in range(9):
                                    for kt in range(4):
                                        w = p3w.tile(
                                            [128, 128], BF16,
                                            tag=f"fwx_{t_i}_{kt}",
                                            name=f"fwx_{ct}_{t_i}_{kt}")
                                        nc.scalar.dma_start(
                                            out=w,
                                            in_=d_fwT[t_i, kt, ct, :, :])
                                        fw[(t_i, kt)] = w
                            for ch in range(8):
                                pf = psA.tile([128, 512], F32, tag="conv",
                                              name=f"pf_{ct}_{ch}")
                                conv3x3_chunk(nc, pf, fw, x65, ch * 8, 4)
                                nc.scalar.activation(
                                    out=yt[ct][:, ch * 512:(ch + 1) * 512],
                                    in_=pf, func=AF.Identity,
                                    bias=fb[:, ct:ct + 1], scale=1.0)
                    # pass 2: agg half, accumulated into yt on DVE
                    for ct in range(4):
                        fw = {(t_i, kt): fwa[(ct, t_i, kt)]
                              for t_i in range(9) for kt in range(4)}
                        for ch in range(8):
                            pf = psA.tile([128, 512], F32, tag="conv",
                                          name=f"pf2_{ct}_{ch}")
                            conv3x3_chunk(nc, pf, fw, agg65, ch * 8, 4)
                            nc.vector.tensor_add(
                                out=yt[ct][:, ch * 512:(ch + 1) * 512],
                                in0=yt[ct][:, ch * 512:(ch + 1) * 512],
                                in1=pf)
                        # per-ct BN partial stats, overlapped with next ct
                        st = tail.tile([128, 8, 6], F32, tag="st",
                                       name=f"st_{ct}")
                        for j in range(8):
                            nc.vector.bn_stats(
                                out=st[:, j, :],
                                in_=yt[ct][:, j * 512:(j + 1) * 512])
                        mv4 = tail.tile([128, 2], F32, tag="mv4",
                                        name=f"mv4_{ct}")
                        nc.vector.bn_aggr(out=mv4, in_=st)
                        nc.scalar.activation(out=stats_l[:, 2 * ct:2 * ct + 1],
                                             in_=mv4[:, 0:1], func=AF.Copy,
                                             bias=0.0, scale=float(SP))
                        sq = tail.tile([128, 1], F32, tag="sq", name=f"sq_{ct}")
                        nc.vector.tensor_mul(out=sq, in0=mv4[:, 0:1],
                                             in1=mv4[:, 0:1])
                        nc.vector.tensor_add(out=sq, in0=sq, in1=mv4[:, 1:2])
                        nc.scalar.activation(out=stats_l[:, 2 * ct + 1:2 * ct + 2],
                                             in_=sq, func=AF.Copy,
                                             bias=0.0, scale=float(SP))

                with tc.tile_pool(name="p4", bufs=1) as p4:
                    bng = p4.tile([128, 4], F32)
                    nc.sync.dma_start(
                        out=bng, in_=d_bng[:].rearrange("(a b) c -> b (a c)", a=4))
                    bnb = p4.tile([128, 4], F32)
                    nc.sync.dma_start(
                        out=bnb, in_=d_bnb[:].rearrange("(a b) c -> b (a c)", a=4))
                    cc_in = dram.tile([128, 8], F32)
                    cc_out = dram.tile([128, 8], F32)
                    nc.gpsimd.dma_start(out=cc_in, in_=stats_l)
                    nc.gpsimd.collective_compute(
                        "AllReduce", mybir.AluOpType.add,
                        replica_groups=[list(range(N_CORES))],
                        ins=[cc_in.opt()], outs=[cc_out.opt()])
                    rstats = p4.tile([128, 8], F32)
                    nc.sync.dma_start(out=rstats, in_=cc_out)
                    eps128 = p4.tile([128, 1], F32)
                    nc.vector.memset(eps128, float(EPS))
                    NTOT = float(B * SP)
                    for ct in range(4):
                        mean = p4.tile([128, 1], F32, tag="mean", name=f"mn_{ct}")
                        nc.scalar.activation(out=mean,
                                             in_=rstats[:, 2 * ct:2 * ct + 1],
                                             func=AF.Copy, bias=0.0,
                                             scale=1.0 / NTOT)
                        var = p4.tile([128, 1], F32, tag="var", name=f"vr_{ct}")
                        nc.vector.tensor_mul(out=var, in0=mean, in1=mean)
                        ex2t = p4.tile([128, 1], F32, tag="ex2t", name=f"e2_{ct}")
                        nc.scalar.activation(out=ex2t,
                                             in_=rstats[:, 2 * ct + 1:2 * ct + 2],
                                             func=AF.Copy, bias=0.0,
                                             scale=1.0 / NTOT)
                        nc.vector.tensor_sub(out=var, in0=ex2t, in1=var)
                        nc.scalar.activation(out=var, in_=var, func=AF.Sqrt,
                                             bias=eps128, scale=1.0)
                        nc.vector.reciprocal(out=var, in_=var)
                        sc = p4.tile([128, 1], F32, tag="sc", name=f"sc_{ct}")
                        nc.vector.tensor_mul(out=sc, in0=var,
                                             in1=bng[:, ct:ct + 1])
                        sh = p4.tile([128, 1], F32, tag="sh", name=f"sh_{ct}")
                        nc.vector.tensor_mul(out=sh, in0=mean, in1=sc)
                        nc.vector.tensor_sub(out=sh, in0=bnb[:, ct:ct + 1],
                                             in1=sh)
                        sg = p4.tile([128, SP], F32, tag="sg", bufs=2,
                                     name=f"sg_{ct}")
                        nc.scalar.activation(out=sg, in_=yt[ct], func=AF.Silu,
                                             bias=sh, scale=sc)
                        oeng = (nc.sync, nc.gpsimd, nc.scalar, nc.sync)[ct]
                        oeng.dma_start(out=d_y[ct * 128:(ct + 1) * 128, :],
                                       in_=sg)

    _split_multi_waits(nc)
    return nc


_PROGRAM = None


def _get_program():
    global _PROGRAM
    if _PROGRAM is None:
        _PROGRAM = build_program()
    return _PROGRAM


def _bf16(a):
    return np.ascontiguousarray(np.asarray(a, np.float32).astype(ml_dtypes.bfloat16))


def _f32(a):
    return np.ascontiguousarray(np.asarray(a, np.float32))


def kernel(x, box_w1, box_b1, box_w2, box_b2, edge_w1, edge_b1, gn_g, gn_b,
           edge_w2, edge_b2, qkv_w, qkv_b, fus_w, fus_b, bn_g, bn_b,
           trace=False):
    global LAST_RESULTS
    x = np.asarray(x, np.float32)
    scale = float(HD) ** -0.5

    qkv_w2 = np.asarray(qkv_w, np.float32).reshape(3 * C, C)
    qkv_b2 = np.asarray(qkv_b, np.float32).copy()
    wq = qkv_w2[0:C] * scale
    bq = qkv_b2[0:C] * scale
    wk, bk = qkv_w2[C:2 * C], qkv_b2[C:2 * C]
    wv, bv_ = qkv_w2[2 * C:], qkv_b2[2 * C:]
    wqk = np.empty((1024, C), np.float32)
    bqk = np.empty(1024, np.float32)
    for h in range(NH):
        wqk[h * 128:h * 128 + 64] = wq[h * 64:(h + 1) * 64]
        wqk[h * 128 + 64:(h + 1) * 128] = wk[h * 64:(h + 1) * 64]
        bqk[h * 128:h * 128 + 64] = bq[h * 64:(h + 1) * 64]
        bqk[h * 128 + 64:(h + 1) * 128] = bk[h * 64:(h + 1) * 64]

    bw1T = np.asarray(box_w1, np.float32).transpose(2, 3, 1, 0).reshape(9, C, 64)
    ew1T = np.asarray(edge_w1, np.float32).transpose(2, 3, 1, 0).reshape(9, 4, 64)
    fwT = np.asarray(fus_w, np.float32).transpose(2, 3, 1, 0).reshape(9, 1024, C)
    fwT_t = np.ascontiguousarray(
        fwT.reshape(9, 8, 128, 4, 128).transpose(0, 1, 3, 2, 4))

    gmat = np.zeros((64, 8), np.float32)
    for g in range(8):
        gmat[g * 8:(g + 1) * 8, g] = 1.0

    shared = {
        "wqkT": _bf16(wqk.T), "wvT": _bf16(wv.T),
        "bqkB": _bf16(bqk[None, :]),
        "bvB": _bf16(bv_[None, :]),
        "bw1": _bf16(bw1T), "bb1": _f32(np.asarray(box_b1).reshape(64, 1)),
        "bw2": _bf16(np.asarray(box_w2, np.float32).reshape(4, 64).T),
        "bb2": _f32(np.asarray(box_b2).reshape(4, 1)),
        "ew1": _bf16(ew1T), "eb1": _f32(np.asarray(edge_b1).reshape(64, 1)),
        "ew2": _bf16(np.asarray(edge_w2, np.float32).reshape(8, 64).T),
        "eb2": _f32(np.asarray(edge_b2).reshape(8, 1)),
        "gng": _f32(np.asarray(gn_g).reshape(64, 1)),
        "gnb": _f32(np.asarray(gn_b).reshape(64, 1)),
        "gmat": gmat,
        "fwT": _bf16(fwT_t),
        "fb": _f32(np.asarray(fus_b).reshape(C, 1)),
        "bng": _f32(np.asarray(bn_g).reshape(C, 1)),
        "bnb": _f32(np.asarray(bn_b).reshape(C, 1)),
    }

    in_maps = []
    for b in range(B):
        xb = x[b]
        x65h = np.zeros((C, 64, W65), np.float32)
        x65h[:, :, 1:] = xb
        x65h = np.concatenate(
            [x65h.reshape(C, 4160), np.zeros((C, 4), np.float32)], axis=1)
        m = dict(shared)
        m["x65"] = _bf16(x65h)
        m["xT"] = _bf16(np.ascontiguousarray(xb.transpose(0, 2, 1)).reshape(C, SP))
        in_maps.append(m)

    nc = _get_program()
    res = run_bass_kernel_spmd(nc, in_maps, core_ids=list(range(N_CORES)),
                               trace=trace)
    LAST_RESULTS = res
    out = np.empty((B, C, H, W), np.float32)
    for b in range(B):
        out[b] = res.results[b]["y"].reshape(C, H, W)
    return out


# revision 52
# speedup vs baseline: 1.1009x; 1.0095x over previous
"""AutoBoxGraphAttention Trainium2 kernel (optimized).

Data-parallel over batch: core b handles image b (B=8, one per NeuronCore).
The only cross-core communication is a 4KB AllReduce of BatchNorm partials.

Key layout/scheduling choices vs the naive version:
  - qkv biases folded into the PSUM->SBUF copies (tensor_tensor add with
    host-precomputed partition-broadcast bias tiles). No K=1 bias matmuls.
  - edge bias folded into the QK^T contraction as a 65th K row: qka has a
    65th partition whose q-slots hold edge[h,i,w] and k-slots hold 1.0.
  - v conv processes two image rows per matmul (N=128, full PE width).
  - S uses transposed free layout S[W', w*64+i] so the softmax reduction
    over i is contiguous; exp() reads PSUM directly (no S copy).
  - agg returns to conv world via SBUF->SBUF DMAs (no DRAM bounce, no
    readback), issue alternating sync/gpsimd queues.
  - fusion conv runs as two passes: the x half (independent of attention)
    right after attention to hide the agg DMA drain, then the agg half
    accumulated into yt with DVE adds.

Spatial layout "w65": each row padded to 65 elements with a LEADING zero
(index d*65 holds 0, data at d*65+1 .. +64, plus 4 trailing zeros; total
4164). A 3x3 tap (dy,dx) then reads a plain strided AP at offset
(r+dy-1)*65 + dx; horizontal SAME-padding is automatic, vertical padding
via per-tap row clamping.
"""

import os
import sys

for _p in ("/opt/trn_rl_repo", os.path.dirname(os.path.abspath(__file__))):
    if _p not in sys.path:
        sys.path.insert(0, _p)

import numpy as np
import ml_dtypes

import concourse.tile as _tile_mod


def _apply_toolchain_patches():
    """This container's walrus accepts at most ONE sync-wait per
    instruction; Tile's exit drain and scheduler attach several. Split the
    exit drain into single-wait drains, and post-process the module to
    hoist extra waits onto same-engine NoOps."""
    import concourse.mybir as mybir

    def _split_drain_and_barrier(self, tick_clock, wait_clock):
        from concourse.tile import ScopedClock
        nc = self.nc
        drain_inst = nc.sync.drain()
        wait_clock.add_sem_waits(
            drain_inst.ins, ScopedClock({None: tick_clock.global_clock}))
        si = drain_inst.ins.sync_info
        if si is not None and len(si.on_wait) > 1:
            waits = list(si.on_wait)
            drain_inst.ins.sync_info = type(si)(
                on_wait=waits[:1], on_update=list(si.on_update))
            for w in waits[1:]:
                d2 = nc.sync.drain()
                si2 = d2.ins.sync_info
                if si2 is None:
                    d2.ins.sync_info = type(si)(on_wait=[w], on_update=[])
                else:
                    d2.ins.sync_info = type(si2)(
                        on_wait=list(si2.on_wait) + [w],
                        on_update=list(si2.on_update))
        nc.all_engine_barrier()
        assert self.sems is not None
        popped = nc._tile_sem_poison_stack.pop()
        assert popped is self._sem_poison
        nc.clear_and_free_semaphores(list(self.sems.allocated().values()))
        nc.all_engine_barrier()

    _tile_mod.TileContext._drain_and_barrier = _split_drain_and_barrier


def _split_multi_waits(nc):
    import concourse.mybir as mybir
    n_split = 0
    for fn in nc.m.functions:
        for bb in fn.blocks:
            insts = list(bb.instructions)
            out = []
            changed = False
            for inst in insts:
                si = inst.sync_info
                if si is not None and len(si.on_wait) > 1:
                    waits = list(si.on_wait)
                    for w in waits[:-1]:
                        nop = mybir.InstNoOp(
                            name=f"{inst.name}-wsplit{n_split}",
                            engine=inst.engine, bass_nofuse=True)
                        nop.sync_info = mybir.SyncInfo(on_wait=[w], on_update=[])
                        out.append(nop)
                        n_split += 1
                    inst.sync_info = type(si)(
                        on_wait=[waits[-1]], on_update=list(si.on_update))
                    changed = True
                out.append(inst)
            if changed:
                bb.instructions = out
    return n_split


_apply_toolchain_patches()

import concourse.bass as bass  # noqa: E402
import concourse.tile as tile  # noqa: E402
from concourse import mybir  # noqa: E402
from concourse.bass_utils import run_bass_kernel_spmd  # noqa: E402

F32 = mybir.dt.float32
BF16 = mybir.dt.bfloat16

B, C, H, W = 8, 512, 64, 64
NH, HD = 8, 64
EPS = 1e-5
W65 = 65
SP65 = 64 * W65 + 4  # 4164
SP = 4096
N_CORES = 8

LAST_RESULTS = None


def _ap(t, offset, dims):
    return bass.AP(tensor=t.tensor, offset=t.offset + offset, ap=[t.ap[0]] + dims)


def _slice_part(t, p0, np_, offset, dims):
    # tile[p0:p0+np_] then rebuild free dims
    sub = t[p0:p0 + np_]
    return bass.AP(tensor=sub.tensor, offset=sub.offset + offset,
                   ap=[sub.ap[0]] + dims)


TAPS = [(1, 0), (1, 1), (1, 2), (0, 0), (0, 1), (0, 2), (2, 0), (2, 1), (2, 2)]


def conv3x3_chunk(nc, psum, w_tiles, src_tiles, r0, n_ktiles, co=128):
    """3x3 conv, one 8-row chunk, accumulated into psum (co, 8*64).
    w_tiles[(tap_idx, kt)] = lhsT (K, co); src_tiles[kt] = w65 tile.
    dy=1 taps first (full coverage -> start=True clears the bank)."""
    mms = []
    for dy, dx in TAPS:
        t_i = dy * 3 + dx  # host weight tap order is (ky, kx) row-major
        a = max(r0, 1 - dy)
        b = min(r0 + 8, 65 - dy)
        nr = b - a
        if nr <= 0:
            continue
        for kt in range(n_ktiles):
            mms.append((t_i, kt, a, nr, dy, dx))
    for j, (t_i, kt, a, nr, dy, dx) in enumerate(mms):
        src = src_tiles[kt]
        in_ap = _ap(src, (a + dy - 1) * W65 + dx, [[W65, nr], [1, 64]])
        nc.tensor.matmul(
            psum[0:co, (a - r0) * 64:(a - r0 + nr) * 64], w_tiles[(t_i, kt)],
            in_ap, start=(j == 0), stop=(j == len(mms) - 1),
            skip_group_check=True)


def build_program():
    nc = bass.Bass(trn_type="TRN2", num_devices=N_CORES)

    d_x65 = nc.dram_tensor("x65", [C, SP65], BF16, kind="ExternalInput")
    d_xT = nc.dram_tensor("xT", [C, SP], BF16, kind="ExternalInput")
    d_wqkT = nc.dram_tensor("wqkT", [C, 1024], BF16, kind="ExternalInput")
    d_wvT = nc.dram_tensor("wvT", [C, 512], BF16, kind="ExternalInput")
    d_bqkB = nc.dram_tensor("bqkB", [1, 1024], BF16, kind="ExternalInput")
    d_bvB = nc.dram_tensor("bvB", [1, 512], BF16, kind="ExternalInput")
    d_bw1 = nc.dram_tensor("bw1", [9, C, 64], BF16, kind="ExternalInput")
    d_bb1 = nc.dram_tensor("bb1", [64, 1], F32, kind="ExternalInput")
    d_bw2 = nc.dram_tensor("bw2", [64, 4], BF16, kind="ExternalInput")
    d_bb2 = nc.dram_tensor("bb2", [4, 1], F32, kind="ExternalInput")
    d_ew1 = nc.dram_tensor("ew1", [9, 4, 64], BF16, kind="ExternalInput")
    d_eb1 = nc.dram_tensor("eb1", [64, 1], F32, kind="ExternalInput")
    d_ew2 = nc.dram_tensor("ew2", [64, 8], BF16, kind="ExternalInput")
    d_eb2 = nc.dram_tensor("eb2", [8, 1], F32, kind="ExternalInput")
    d_gng = nc.dram_tensor("gng", [64, 1], F32, kind="ExternalInput")
    d_gnb = nc.dram_tensor("gnb", [64, 1], F32, kind="ExternalInput")
    d_gmat = nc.dram_tensor("gmat", [64, 8], F32, kind="ExternalInput")
    d_fwT = nc.dram_tensor("fwT", [9, 8, 4, 128, 128], BF16, kind="ExternalInput")
    d_fb = nc.dram_tensor("fb", [C, 1], F32, kind="ExternalInput")
    d_bng = nc.dram_tensor("bng", [C, 1], F32, kind="ExternalInput")
    d_bnb = nc.dram_tensor("bnb", [C, 1], F32, kind="ExternalInput")
    d_y = nc.dram_tensor("y", [C, SP], F32, kind="ExternalOutput")

    AF = mybir.ActivationFunctionType

    with tile.TileContext(nc) as tc:
        with tc.tile_pool(name="glob", bufs=1) as glob, \
             tc.tile_pool(name="psA", bufs=4, space="PSUM") as psA, \
             tc.tile_pool(name="psB", bufs=4, space="PSUM") as psB, \
             tc.tile_pool(name="dram", bufs=1, space="DRAM") as dram:

            x65 = [glob.tile([128, SP65], BF16, name=f"x65_{k}")
                   for k in range(4)]
            agg65 = [glob.tile([128, SP65], BF16, name=f"agg65_{k}")
                     for k in range(4)]
            ones1 = glob.tile([1, 128], BF16)
            fb = glob.tile([128, 4], F32)
            nc.sync.dma_start(
                out=fb, in_=d_fb[:].rearrange("(a b) c -> b (a c)", a=4))
            for k in range(4):
                nc.sync.dma_start(out=x65[k], in_=d_x65[k * 128:(k + 1) * 128, :])
                nc.gpsimd.memset(agg65[k], 0.0)
            nc.vector.memset(ones1, 1.0)
            fw0 = {}
            for t_i in range(9):
                for kt in range(4):
                    fw0[(t_i, kt)] = glob.tile([128, 128], BF16,
                                               name=f"fw0_{t_i}_{kt}")

            # ================= Phases 1-2 pool =================
            with tc.tile_pool(name="ph12", bufs=1) as ph12:
                xT = [ph12.tile([128, SP], BF16, name=f"xT_{k}")
                      for k in range(4)]
                wqkT = [ph12.tile([128, 1024], BF16, name=f"wqkT_{kt}")
                        for kt in range(4)]
                wvT = [ph12.tile([128, 512], BF16, name=f"wvT_{kt}")
                       for kt in range(4)]
                for kt in range(4):
                    nc.gpsimd.dma_start(out=wqkT[kt],
                                        in_=d_wqkT[kt * 128:(kt + 1) * 128, :])
                    nc.gpsimd.dma_start(out=wvT[kt],
                                        in_=d_wvT[kt * 128:(kt + 1) * 128, :])
                bqkB = ph12.tile([1, 1024], BF16)
                nc.gpsimd.dma_start(out=bqkB, in_=d_bqkB[:])
                bvB = ph12.tile([1, 512], BF16)
                nc.gpsimd.dma_start(out=bvB, in_=d_bvB[:])
                edge_flT = ph12.tile([8, SP], BF16)

                # ---------- Phase 1: box_net + edge_net ----------
                with tc.tile_pool(name="p1", bufs=1) as p1:
                    for k in range(4):
                        nc.gpsimd.dma_start(out=xT[k],
                                            in_=d_xT[k * 128:(k + 1) * 128, :])
                    bw1 = {}
                    for t_i in range(9):
                        for kt in range(4):
                            w = p1.tile([128, 64], BF16, name=f"bw1_{t_i}_{kt}")
                            nc.sync.dma_start(
                                out=w, in_=d_bw1[t_i, kt * 128:(kt + 1) * 128, :])
                            bw1[(t_i, kt)] = w
                    bb1 = p1.tile([64, 1], F32)
                    nc.sync.dma_start(out=bb1, in_=d_bb1[:])
                    bw2 = p1.tile([64, 4], BF16)
                    nc.sync.dma_start(out=bw2, in_=d_bw2[:])
                    bb2 = p1.tile([4, 1], F32)
                    nc.sync.dma_start(out=bb2, in_=d_bb2[:])
                    ew1 = {}
                    for t_i in range(9):
                        w = p1.tile([4, 64], BF16, name=f"ew1_{t_i}")
                        nc.sync.dma_start(out=w, in_=d_ew1[t_i, :, :])
                        ew1[(t_i, 0)] = w
                    eb1 = p1.tile([64, 1], F32)
                    nc.sync.dma_start(out=eb1, in_=d_eb1[:])
                    ew2 = p1.tile([64, 8], BF16)
                    nc.sync.dma_start(out=ew2, in_=d_ew2[:])
                    eb2 = p1.tile([8, 1], F32)
                    nc.sync.dma_start(out=eb2, in_=d_eb2[:])
                    gng = p1.tile([64, 1], F32)
                    nc.sync.dma_start(out=gng, in_=d_gng[:])
                    gnb = p1.tile([64, 1], F32)
                    nc.sync.dma_start(out=gnb, in_=d_gnb[:])
                    gmat = p1.tile([64, 8], F32)
                    nc.sync.dma_start(out=gmat, in_=d_gmat[:])
                    for (t_i, kt), w in fw0.items():
                        nc.sync.dma_start(out=w, in_=d_fwT[t_i, kt, 0, :, :])

                    box1 = p1.tile([64, SP65], BF16)
                    nc.vector.memset(box1, 0.0)
                    for ch in range(8):
                        pb = psA.tile([128, 512], F32, tag="conv", name=f"pb_{ch}")
                        conv3x3_chunk(nc, pb, bw1, x65, ch * 8, 4, co=64)
                        nc.scalar.activation(
                            out=_slice_part(box1, 0, 64, ch * 8 * W65 + 1,
                                            [[W65, 8], [1, 64]]),
                            in_=pb[0:64, :], func=AF.Gelu, bias=bb1, scale=1.0)

                    boxes = p1.tile([4, SP65], BF16)
                    nc.vector.memset(boxes, 0.0)
                    for ch in range(8):
                        pb2 = psA.tile([128, 512], F32, tag="conv", name=f"pb2_{ch}")
                        nc.tensor.matmul(
                            pb2[0:4, :], bw2,
                            _ap(box1, ch * 8 * W65 + 1, [[W65, 8], [1, 64]]),
                            start=True, stop=True)
                        nc.scalar.activation(
                            out=_slice_part(boxes, 0, 4, ch * 8 * W65 + 1,
                                            [[W65, 8], [1, 64]]),
                            in_=pb2[0:4, :], func=AF.Sigmoid, bias=bb2, scale=1.0)

                    e1 = p1.tile([64, SP], F32)
                    for ch in range(8):
                        pe = psA.tile([128, 512], F32, tag="conv", name=f"pe_{ch}")
                        conv3x3_chunk(nc, pe, ew1, [boxes], ch * 8, 1, co=64)
                        nc.scalar.activation(
                            out=e1[:, ch * 512:(ch + 1) * 512], in_=pe[0:64, :],
                            func=AF.Identity, bias=eb1, scale=1.0)

                    stats = p1.tile([64, 8, 6], F32)
                    for j in range(8):
                        nc.vector.bn_stats(out=stats[:, j, :],
                                           in_=e1[:, j * 512:(j + 1) * 512])
                    mv = p1.tile([64, 2], F32)
                    nc.vector.bn_aggr(out=mv, in_=stats)
                    ex2 = p1.tile([64, 2], F32)
                    nc.vector.tensor_copy(out=ex2[:, 0:1], in_=mv[:, 0:1])
                    nc.vector.tensor_mul(out=ex2[:, 1:2], in0=mv[:, 0:1],
                                         in1=mv[:, 0:1])
                    nc.vector.tensor_add(out=ex2[:, 1:2], in0=ex2[:, 1:2],
                                         in1=mv[:, 1:2])
                    gs_ps = psB.tile([8, 2], F32, tag="att", name="gs_ps")
                    nc.tensor.matmul(gs_ps, gmat, ex2, start=True, stop=True)
                    gs = p1.tile([8, 2], F32)
                    nc.scalar.activation(out=gs, in_=gs_ps, func=AF.Copy,
                                         bias=0.0, scale=1.0 / 8.0)
                    gvar = p1.tile([8, 1], F32)
                    eps8 = p1.tile([8, 1], F32)
                    nc.vector.memset(eps8, float(EPS))
                    nc.vector.tensor_mul(out=gvar, in0=gs[:, 0:1], in1=gs[:, 0:1])
                    nc.vector.tensor_sub(out=gvar, in0=gs[:, 1:2], in1=gvar)
                    nc.scalar.activation(out=gvar, in_=gvar, func=AF.Sqrt,
                                         bias=eps8, scale=1.0)
                    nc.vector.reciprocal(out=gvar, in_=gvar)
                    gmr = p1.tile([8, 2], F32)
                    nc.vector.tensor_copy(out=gmr[:, 0:1], in_=gs[:, 0:1])
                    nc.vector.tensor_copy(out=gmr[:, 1:2], in_=gvar)
                    cmr = p1.tile([64, 2], F32)
                    src_bc = bass.AP(tensor=gmr.tensor, offset=gmr.offset,
                                     ap=[[gmr.ap[0][0], 8], [0, 8], [1, 2]])
                    nc.sync.dma_start(out=cmr, in_=src_bc)
                    gsc = p1.tile([64, 1], F32)
                    nc.vector.tensor_mul(out=gsc, in0=cmr[:, 1:2], in1=gng)
                    gsh = p1.tile([64, 1], F32)
                    nc.vector.tensor_mul(out=gsh, in0=cmr[:, 0:1], in1=gsc)
                    nc.vector.tensor_sub(out=gsh, in0=gnb, in1=gsh)
                    e1g = p1.tile([64, SP], BF16)
                    nc.scalar.activation(out=e1g, in_=e1, func=AF.Gelu,
                                         bias=gsh, scale=gsc)
                    # edge conv2 with transposed moving AP -> edge_flT[h, w*64+i]
                    for wb in range(8):
                        pe2 = psA.tile([128, 512], F32, tag="conv",
                                       name=f"pe2_{wb}")
                        nc.tensor.matmul(
                            pe2[0:8, :], ew2,
                            _ap(e1g, wb * 8, [[1, 8], [64, 64]]),
                            start=True, stop=True)
                        nc.scalar.activation(
                            out=edge_flT[:, wb * 512:(wb + 1) * 512],
                            in_=pe2[0:8, :], func=AF.Identity, bias=eb2,
                            scale=1.0)

                # ---------- Phase 2: qkv conv + attention ----------
                with tc.tile_pool(name="p2", bufs=1) as p2:
                    n_agg_dma = 0
                    n_drain = 0

                    def drain(out, in_):
                        nonlocal n_drain
                        n_drain += 1
                        if n_drain % 2:
                            nc.vector.tensor_copy(out=out, in_=in_)
                        else:
                            nc.scalar.activation(out=out, in_=in_,
                                                 func=AF.Copy, bias=0.0,
                                                 scale=1.0)

                    for g in range(4):
                        qka = p2.tile([65, 64 * 256], BF16, tag="qka",
                                      name=f"qka_{g}")
                        va = p2.tile([64, 64 * 128], BF16, tag="va",
                                     name=f"va_{g}")
                        # qk conv: 2 column-pairs per psum group, PE bias row
                        for spp in range(16):
                            pq = psA.tile([128, 512], F32, tag="conv",
                                          name=f"pq_{g}_{spp}")
                            for q in range(2):
                                sp = 2 * spp + q
                                for kt in range(4):
                                    nc.tensor.matmul(
                                        pq[:, q * 256:(q + 1) * 256],
                                        xT[kt][:, sp * 128:(sp + 1) * 128],
                                        wqkT[kt][:, g * 256:(g + 1) * 256],
                                        start=(q == 0 and kt == 0),
                                        stop=False,
                                        skip_group_check=True)
                            nc.tensor.matmul(
                                pq, ones1,
                                _slice_part(bqkB, 0, 1, g * 256,
                                            [[0, 2], [1, 256]]),
                                start=False, stop=True,
                                skip_group_check=True)
                            for j in range(2):
                                drain(
                                    _slice_part(qka, 0, 64, (4 * spp + j) * 256,
                                                [[512, 2], [1, 256]]),
                                    pq[64 * j:64 * (j + 1), :])
                        # qka 65th row: k-slots = 1.0, q-slots = edge[h,i,w]
                        nc.scalar.dma_start(
                            out=_slice_part(qka, 64, 1, 64,
                                            [[256, 64], [128, 2], [1, 64]]),
                            in_=_slice_part(ones1, 0, 1, 0, [[0, 128], [1, 64]]))
                        for hh in range(2):
                            nc.scalar.dma_start(
                                out=_slice_part(qka, 64, 1, hh * 128,
                                                [[256, 64], [1, 64]]),
                                in_=_slice_part(edge_flT, 2 * g + hh, 1, 0,
                                                [[64, 64], [1, 64]]))
                        # QK^T logits (K=65 incl. edge row), exp from psum
                        Sx = [None, None]
                        for hh in range(2):
                            qoff, koff = hh * 128, hh * 128 + 64
                            Sx[hh] = p2.tile([64, SP], BF16, tag=f"Sx{hh}",
                                             name=f"Sx_{g}_{hh}")
                            for ib in range(8):
                                sp_ = psB.tile([64, 512], F32, tag="att",
                                               name=f"sp_{g}_{hh}_{ib}")
                                for ii in range(8):
                                    i = ib * 8 + ii
                                    out_ap = bass.AP(
                                        tensor=sp_.tensor,
                                        offset=sp_.offset + ii,
                                        ap=[sp_.ap[0], [8, 64]])
                                    nc.tensor.matmul(
                                        out_ap,
                                        _ap(qka, koff + i, [[256, 64]]),
                                        _ap(qka, qoff + i, [[256, 64]]),
                                        start=(ii == 0), stop=(ii == 7),
                                        skip_group_check=True)
                                nc.scalar.activation(
                                    out=_ap(Sx[hh], ib * 8,
                                            [[64, 64], [1, 8]]),
                                    in_=sp_, func=AF.Exp)
                        # v conv: 4 rows per psum group (x65 rows
                        # stationary); fills the PE while exp/softmax run
                        for dq in range(16):
                            pv = psA.tile([64, 512], F32, tag="conv",
                                          name=f"pv_{g}_{dq}")
                            for q in range(4):
                                d0 = 4 * dq + q
                                for kt in range(4):
                                    lhs = _ap(x65[kt], d0 * W65 + 1, [[1, 64]])
                                    nc.tensor.matmul(
                                        pv[:, q * 128:(q + 1) * 128], lhs,
                                        wvT[kt][:, g * 128:(g + 1) * 128],
                                        start=(q == 0 and kt == 0),
                                        stop=False,
                                        skip_group_check=True)
                            nc.tensor.matmul(
                                pv, ones1[:, 0:64],
                                _slice_part(bvB, 0, 1, g * 128,
                                            [[0, 4], [1, 128]]),
                                start=False, stop=True,
                                skip_group_check=True)
                            drain(_ap(va, 4 * dq * 128, [[1, 512]]), pv)
                        # softmax over i (contiguous), both heads first
                        for hh in range(2):
                            D = p2.tile([64, 64], F32, tag=f"D{hh}",
                                        name=f"D_{g}_{hh}")
                            Pv = bass.AP(tensor=Sx[hh].tensor,
                                         offset=Sx[hh].offset,
                                         ap=[Sx[hh].ap[0], [64, 64], [1, 64]])
                            nc.vector.reduce_sum(out=D, in_=Pv,
                                                 axis=mybir.AxisListType.X)
                            nc.vector.reciprocal(out=D, in_=D)
                            Rb = bass.AP(tensor=D.tensor, offset=D.offset,
                                         ap=[D.ap[0], [1, 64], [0, 64]])
                            nc.vector.tensor_mul(out=Sx[hh], in0=Sx[hh],
                                                 in1=Rb)
                        # AV per head
                        for hh in range(2):
                            h = 2 * g + hh
                            for ib in range(8):
                                ap2 = psB.tile([64, 512], F32, tag="att",
                                               name=f"ap2_{g}_{hh}_{ib}")
                                for ii in range(8):
                                    i = ib * 8 + ii
                                    nc.tensor.matmul(
                                        ap2[:, ii * 64:(ii + 1) * 64],
                                        _ap(va, hh * 64 + i, [[128, 64]]),
                                        _ap(Sx[hh], i, [[64, 64]]),
                                        start=(ii == 0), stop=(ii == 7),
                                        skip_group_check=True)
                                blk = glob.tile([64, 512], BF16, tag="blk",
                                                bufs=8,
                                                name=f"blk_{g}_{hh}_{ib}")
                                drain(blk, ap2)
                                for ii in range(8):
                                    i = ib * 8 + ii
                                    c = h * 64 + i
                                    kt, p = c // 128, c % 128
                                    rot = (nc.gpsimd, nc.sync, nc.gpsimd)
                                    eng = rot[n_agg_dma % 3]
                                    n_agg_dma += 1
                                    eng.dma_start(
                                        out=_slice_part(
                                            agg65[kt], p, 1, 1,
                                            [[W65, 64], [1, 64]]),
                                        in_=blk[:, ii * 64:(ii + 1) * 64])

            # ================= Phases 3-4 pool =================
            with tc.tile_pool(name="tail", bufs=1) as tail:
                yt = [tail.tile([128, SP], F32, name=f"y_{k}") for k in range(4)]
                stats_l = tail.tile([128, 8], F32)
                # pass-2 weights preload on sync at tail-open; pass-1's
                # rotating pool nests inside so their spaces are disjoint
                with tc.tile_pool(name="p3w2", bufs=1) as p3w2:
                    fwa = {}
                    for ct in range(4):
                        for t_i in range(9):
                            for kt in range(4):
                                w = p3w2.tile([128, 128], BF16,
                                              name=f"fwa_{ct}_{t_i}_{kt}")
                                nc.sync.dma_start(
                                    out=w, in_=d_fwT[t_i, kt + 4, ct, :, :])
                                fwa[(ct, t_i, kt)] = w
                    # pass 1: x half of the fusion conv
                    with tc.tile_pool(name="p3w", bufs=2) as p3w:
                        for ct in range(4):
                            if ct == 0:
                                fw = fw0
                            else:
                                fw = {}
                                for t_i in range(9):
                                    for kt in range(4):
                                        w = p3w.tile(
                                            [128, 128], BF16,
                                            tag=f"fwx_{t_i}_{kt}",
                                            name=f"fwx_{ct}_{t_i}_{kt}")
                                        nc.scalar.dma_start(
                                            out=w,
                                            in_=d_fwT[t_i, kt, ct, :, :])
                                        fw[(t_i, kt)] = w
                            for ch in range(8):
                                pf = psA.tile([128, 512], F32, tag="conv",
                                              name=f"pf_{ct}_{ch}")
                                conv3x3_chunk(nc, pf, fw, x65, ch * 8, 4)
                                nc.scalar.activation(
                                    out=yt[ct][:, ch * 512:(ch + 1) * 512],
                                    in_=pf, func=AF.Identity,
                                    bias=fb[:, ct:ct + 1], scale=1.0)
                    # pass 2: agg half, accumulated into yt on DVE
                    for ct in range(4):
                        fw = {(t_i, kt): fwa[(ct, t_i, kt)]
                              for t_i in range(9) for kt in range(4)}
                        for ch in range(8):
                            pf = psA.tile([128, 512], F32, tag="conv",
                                          name=f"pf2_{ct}_{ch}")
                            conv3x3_chunk(nc, pf, fw, agg65, ch * 8, 4)
                            nc.vector.tensor_add(
                                out=yt[ct][:, ch * 512:(ch + 1) * 512],
                                in0=yt[ct][:, ch * 512:(ch + 1) * 512],
                                in1=pf)
                        # per-ct BN partial stats, overlapped with next ct
                        st = tail.tile([128, 8, 6], F32, tag="st",
                                       name=f"st_{ct}")
                        for j in range(8):
                            nc.vector.bn_stats(
                                out=st[:, j, :],
                                in_=yt[ct][:, j * 512:(j + 1) * 512])
                        mv4 = tail.tile([128, 2], F32, tag="mv4",
                                        name=f"mv4_{ct}")
                        nc.vector.bn_aggr(out=mv4, in_=st)
                        nc.scalar.activation(out=stats_l[:, 2 * ct:2 * ct + 1],
                                             in_=mv4[:, 0:1], func=AF.Copy,
                                             bias=0.0, scale=float(SP))
                        sq = tail.tile([128, 1], F32, tag="sq", name=f"sq_{ct}")
                        nc.vector.tensor_mul(out=sq, in0=mv4[:, 0:1],
                                             in1=mv4[:, 0:1])
                        nc.vector.tensor_add(out=sq, in0=sq, in1=mv4[:, 1:2])
                        nc.scalar.activation(out=stats_l[:, 2 * ct + 1:2 * ct + 2],
                                             in_=sq, func=AF.Copy,
                                             bias=0.0, scale=float(SP))

                with tc.tile_pool(name="p4", bufs=1) as p4:
                    bng = p4.tile([128, 4], F32)
                    nc.sync.dma_start(
                        out=bng, in_=d_bng[:].rearrange("(a b) c -> b (a c)", a=4))
                    bnb = p4.tile([128, 4], F32)
                    nc.sync.dma_start(
                        out=bnb, in_=d_bnb[:].rearrange("(a b) c -> b (a c)", a=4))
                    cc_in = dram.tile([128, 8], F32)
                    cc_out = dram.tile([128, 8], F32)
                    nc.gpsimd.dma_start(out=cc_in, in_=stats_l)
                    nc.gpsimd.collective_compute(
                        "AllReduce", mybir.AluOpType.add,
                        replica_groups=[list(range(N_CORES))],
                        ins=[cc_in.opt()], outs=[cc_out.opt()])
                    rstats = p4.tile([128, 8], F32)
                    nc.sync.dma_start(out=rstats, in_=cc_out)
                    eps128 = p4.tile([128, 1], F32)
                    nc.vector.memset(eps128, float(EPS))
                    NTOT = float(B * SP)
                    # batched normalize constants for all 4 channel tiles
                    meanv = p4.tile([128, 4], F32)
                    nc.scalar.activation(
                        out=meanv,
                        in_=_ap(rstats, 0, [[2, 4]]),
                        func=AF.Copy, bias=0.0, scale=1.0 / NTOT)
                    ex2v = p4.tile([128, 4], F32)
                    nc.scalar.activation(
                        out=ex2v,
                        in_=_ap(rstats, 1, [[2, 4]]),
                        func=AF.Copy, bias=0.0, scale=1.0 / NTOT)
                    varv = p4.tile([128, 4], F32)
                    nc.vector.tensor_mul(out=varv, in0=meanv, in1=meanv)
                    nc.vector.tensor_sub(out=varv, in0=ex2v, in1=varv)
                    nc.scalar.activation(out=varv, in_=varv, func=AF.Sqrt,
                                         bias=eps128, scale=1.0)
                    nc.vector.reciprocal(out=varv, in_=varv)
                    scv = p4.tile([128, 4], F32)
                    nc.vector.tensor_mul(out=scv, in0=varv, in1=bng)
                    shv = p4.tile([128, 4], F32)
                    nc.vector.tensor_mul(out=shv, in0=meanv, in1=scv)
                    nc.vector.tensor_sub(out=shv, in0=bnb, in1=shv)
                    for ct in range(4):
                        sg = p4.tile([128, SP], F32, tag="sg", bufs=2,
                                     name=f"sg_{ct}")
                        nc.scalar.activation(out=sg, in_=yt[ct], func=AF.Silu,
                                             bias=shv[:, ct:ct + 1],
                                             scale=scv[:, ct:ct + 1])
                        oeng = (nc.sync, nc.gpsimd, nc.scalar, nc.sync)[ct]
                        oeng.dma_start(out=d_y[ct * 128:(ct + 1) * 128, :],
                                       in_=sg)

    _split_multi_waits(nc)
    return nc


_PROGRAM = None


def _get_program():
    global _PROGRAM
    if _PROGRAM is None:
        _PROGRAM = build_program()
    return _PROGRAM


def _bf16(a):
    return np.ascontiguousarray(np.asarray(a, np.float32).astype(ml_dtypes.bfloat16))


def _f32(a):
    return np.ascontiguousarray(np.asarray(a, np.float32))


def kernel(x, box_w1, box_b1, box_w2, box_b2, edge_w1, edge_b1, gn_g, gn_b,
           edge_w2, edge_b2, qkv_w, qkv_b, fus_w, fus_b, bn_g, bn_b,
           trace=False):
    global LAST_RESULTS
    x = np.asarray(x, np.float32)
    scale = float(HD) ** -0.5

    qkv_w2 = np.asarray(qkv_w, np.float32).reshape(3 * C, C)
    qkv_b2 = np.asarray(qkv_b, np.float32).copy()
    wq = qkv_w2[0:C] * scale
    bq = qkv_b2[0:C] * scale
    wk, bk = qkv_w2[C:2 * C], qkv_b2[C:2 * C]
    wv, bv_ = qkv_w2[2 * C:], qkv_b2[2 * C:]
    wqk = np.empty((1024, C), np.float32)
    bqk = np.empty(1024, np.float32)
    for h in range(NH):
        wqk[h * 128:h * 128 + 64] = wq[h * 64:(h + 1) * 64]
        wqk[h * 128 + 64:(h + 1) * 128] = wk[h * 64:(h + 1) * 64]
        bqk[h * 128:h * 128 + 64] = bq[h * 64:(h + 1) * 64]
        bqk[h * 128 + 64:(h + 1) * 128] = bk[h * 64:(h + 1) * 64]

    bw1T = np.asarray(box_w1, np.float32).transpose(2, 3, 1, 0).reshape(9, C, 64)
    ew1T = np.asarray(edge_w1, np.float32).transpose(2, 3, 1, 0).reshape(9, 4, 64)
    fwT = np.asarray(fus_w, np.float32).transpose(2, 3, 1, 0).reshape(9, 1024, C)
    fwT_t = np.ascontiguousarray(
        fwT.reshape(9, 8, 128, 4, 128).transpose(0, 1, 3, 2, 4))

    gmat = np.zeros((64, 8), np.float32)
    for g in range(8):
        gmat[g * 8:(g + 1) * 8, g] = 1.0

    shared = {
        "wqkT": _bf16(wqk.T), "wvT": _bf16(wv.T),
        "bqkB": _bf16(bqk[None, :]),
        "bvB": _bf16(bv_[None, :]),
        "bw1": _bf16(bw1T), "bb1": _f32(np.asarray(box_b1).reshape(64, 1)),
        "bw2": _bf16(np.asarray(box_w2, np.float32).reshape(4, 64).T),
        "bb2": _f32(np.asarray(box_b2).reshape(4, 1)),
        "ew1": _bf16(ew1T), "eb1": _f32(np.asarray(edge_b1).reshape(64, 1)),
        "ew2": _bf16(np.asarray(edge_w2, np.float32).reshape(8, 64).T),
        "eb2": _f32(np.asarray(edge_b2).reshape(8, 1)),
        "gng": _f32(np.asarray(gn_g).reshape(64, 1)),
        "gnb": _f32(np.asarray(gn_b).reshape(64, 1)),
        "gmat": gmat,
        "fwT": _bf16(fwT_t),
        "fb": _f32(np.asarray(fus_b).reshape(C, 1)),
        "bng": _f32(np.asarray(bn_g).reshape(C, 1)),
        "bnb": _f32(np.asarray(bn_b).reshape(C, 1)),
    }

    in_maps = []
    for b in range(B):
        xb = x[b]
        x65h = np.zeros((C, 64, W65), np.float32)
        x65h[:, :, 1:] = xb
        x65h = np.concatenate(
            [x65h.reshape(C, 4160), np.zeros((C, 4), np.float32)], axis=1)
        m = dict(shared)
        m["x65"] = _bf16(x65h)
        m["xT"] = _bf16(np.ascontiguousarray(xb.transpose(0, 2, 1)).reshape(C, SP))
        in_maps.append(m)

    nc = _get_program()
    res = run_bass_kernel_spmd(nc, in_maps, core_ids=list(range(N_CORES)),
                               trace=trace)
    LAST_RESULTS = res
    out = np.empty((B, C, H, W), np.float32)
    for b in range(B):
        out[b] = res.results[b]["y"].reshape(C, H, W)
    return out
